# revision 18
# baseline (speedup 1.0000x reference)
"""Multi-head attention with RoPE (B=2, S=2048, H=16, D=64) on 8 TRN2 cores,
tensor-parallel over heads (2 heads/core); host sums the 8 rank-128 fp16
partial outputs.

Restructured from the 252us baseline around two engine budgets:
  - ScalarE runs ONLY the 128 softmax exps ([128,1024] each, ~1.1us);
    everything else (copies, DMA issue) lives on DVE/Sync/GpSimd queues.
  - PE work cut via tile_position packing:
      * scores: h0 (rows 0-63) and h1 (rows 64-127) issue as adjacent
        K=64 matmuls on disjoint row-groups -> run concurrently.
      * PV: v has no ones column (M=64); h0 writes ctx psum partitions
        0-63, h1 partitions 64-127 (col tile (0,64)), concurrently from
        the two p streams.  start=True clears has_written for the whole
        bank but only on the written partitions, so partition-disjoint
        groups are safe (free-offset-disjoint groups are NOT - see
        v_chunk, which uses a single accumulation group per bank).
      * softmax denominators via an "l-quad": four M=1 ones-stationary
        matmuls col-tiled at psum partitions 0/32/64/96, accumulating
        sum_k exp over the kb loop.
  - Single fused timeline: lead-in computes k(b0), rope-k, q(b0) first
    half, rope-q half0, v(b0) chunk0; everything else (rest of b0,
    qkv(b1), output projection) drains from a deadline-tagged work queue
    inside the attention kb loops, paced to fill PE slack under the
    ScalarE exp cadence.
  - PSUM (8 banks): sp0 2 + sp1 2 + packed ctx 2 + l 1 + one shared slot
    (qkv-psum / v-psum / proj) 1.
  - x cached in SBUF per batch (1 HBM read), loaded in [128,512] pieces
    token-major so the first k-projection starts after ~1 piece.
  - outputs stored fp16 (host sums in fp32).
"""
import numpy as np
import ml_dtypes

import concourse.bass as bass
import concourse.mybir as mybir
import concourse.tile as tile
from concourse import bacc
from concourse.bass_utils import run_bass_kernel_spmd

F32 = mybir.dt.float32
F16 = mybir.dt.float16

B, S, HID = 2, 2048, 1024
NH, HD = 16, 64
T = B * S
NCORES = 8
HPC = NH // NCORES         # 2 heads per core
DPC = HPC * HD             # 128 context dims per core
ROPE_BASE = 10000.0

_CACHE = {}
DEBUG_TAPS = False


def _build_program():
    nc = bacc.Bacc("TRN2", target_bir_lowering=False, debug=False)

    xT_d = nc.dram_tensor("xT16", [HID, T], F16, kind="ExternalInput")
    wq_d = nc.dram_tensor("wq", [128, HID], F16, kind="ExternalInput")
    wk_d = nc.dram_tensor("wk", [128, HID], F16, kind="ExternalInput")
    wv_d = nc.dram_tensor("wv", [128, HID], F16, kind="ExternalInput")
    wo_d = nc.dram_tensor("wo", [DPC, HID], F16, kind="ExternalInput")
    cos_d = nc.dram_tensor("cosf", [128, S], F16, kind="ExternalInput")
    sin_d = nc.dram_tensor("sins", [128, S], F16, kind="ExternalInput")
    out_d = nc.dram_tensor("out", [T, HID], F16, kind="ExternalOutput")
    rscr_d = nc.dram_tensor("rscr", [4, 128, 512], F32,
                            kind="ExternalOutput" if DEBUG_TAPS else "Internal")

    dbg = None
    if DEBUG_TAPS:
        dbg = {
            "qT": nc.dram_tensor("dbg_qT", [128, T], F32, kind="ExternalOutput"),
            "kT": nc.dram_tensor("dbg_kT", [128, T], F32, kind="ExternalOutput"),
            "v0": nc.dram_tensor("dbg_v0", [128, 64], F32, kind="ExternalOutput"),
            "p0": nc.dram_tensor("dbg_p0", [128, 1024], F32, kind="ExternalOutput"),
            "lq": nc.dram_tensor("dbg_lq", [128, 512], F32, kind="ExternalOutput"),
            "ls2": nc.dram_tensor("dbg_ls2", [128, 512], F32, kind="ExternalOutput"),
            "lq2": nc.dram_tensor("dbg_lq2", [128, 512], F32, kind="ExternalOutput"),
            "ctx0": nc.dram_tensor("dbg_ctx0", [128, S], F32, kind="ExternalOutput"),
            "vall": nc.dram_tensor("dbg_vall", [128, 64 * HD], F32, kind="ExternalOutput"),
            "ctxq1": nc.dram_tensor("dbg_ctxq1", [128, 1024], F32, kind="ExternalOutput"),
            "bct1": nc.dram_tensor("dbg_bct1", [128, 1024], F32, kind="ExternalOutput"),
        }

    with tile.TileContext(nc) as tc:
        _emit(nc, tc, xT_d, wq_d, wk_d, wv_d, wo_d, cos_d, sin_d, out_d,
              rscr_d, dbg=dbg)
    nc.compile()
    return nc


def _emit(nc, tc, xT_d, wq_d, wk_d, wv_d, wo_d, cos_d, sin_d, out_d,
          rscr_d, dbg=None):
    import contextlib
    ctx = contextlib.ExitStack()
    with ctx:
        singles = ctx.enter_context(tc.tile_pool(name="singles", bufs=1))
        xpool = ctx.enter_context(tc.tile_pool(name="xpool", bufs=2))
        ppool = ctx.enter_context(tc.tile_pool(name="ppool", bufs=3))
        rotp = ctx.enter_context(tc.tile_pool(name="rotp", bufs=3))
        lpool = ctx.enter_context(tc.tile_pool(name="lpool", bufs=2))
        bpool = ctx.enter_context(tc.tile_pool(name="bpool", bufs=2))
        opool = ctx.enter_context(tc.tile_pool(name="opool", bufs=3))
        # PSUM: exactly 8 banks
        spp = ctx.enter_context(tc.tile_pool(name="spp", bufs=1, space="PSUM"))
        ctxp = ctx.enter_context(tc.tile_pool(name="ctxp", bufs=1, space="PSUM"))
        lpp = ctx.enter_context(tc.tile_pool(name="lpp", bufs=1, space="PSUM"))
        shp = ctx.enter_context(tc.tile_pool(name="shp", bufs=1, space="PSUM"))

        # ---- persistent SBUF ----
        wq_sb = singles.tile([128, 8, DPC], F16)
        wk_sb = singles.tile([128, 8, DPC], F16)
        wv_sb = singles.tile([128, 8, DPC], F16)
        wo_sb = singles.tile([128, HID], F16)
        cos_sb = singles.tile([128, S], F16)
        sin_sb = singles.tile([128, S], F16)
        qT_sb = singles.tile([128, T], F16)
        kT_sb = singles.tile([128, T], F16)
        v_all = singles.tile([128, 64, HD], F16)   # slot = h*32 + b*16 + kb
        ctx0_sb = singles.tile([128, S], F16)      # normalized ctx^T, b=0
        ctx1_sb = singles.tile([128, S], F16)
        ones_sb = singles.tile([128, 32], F16)

        nc.sync.dma_start(out=wq_sb[:].rearrange("p a b -> p (a b)"), in_=wq_d[:])
        nc.scalar.dma_start(out=wk_sb[:].rearrange("p a b -> p (a b)"), in_=wk_d[:])
        nc.gpsimd.dma_start(out=wv_sb[:].rearrange("p a b -> p (a b)"), in_=wv_d[:])
        nc.gpsimd.dma_start(out=wo_sb[:], in_=wo_d[:])
        nc.scalar.dma_start(out=cos_sb[:], in_=cos_d[:])
        nc.scalar.dma_start(out=sin_sb[:], in_=sin_d[:])
        nc.vector.memset(ones_sb[:], 1.0)
        # preload the exp table set during the lead-in
        tblw = singles.tile([1, 8], F32)
        nc.vector.memset(tblw[:], 0.0)
        nc.scalar.activation(out=tblw[:], in_=tblw[:],
                             func=mybir.ActivationFunctionType.Exp)

        # x cache: one SBUF slot per batch, [128, kc, tokens-in-batch],
        # loaded token-major in [128,512] pieces so consumers start early
        x_sb = {}

        def load_x(b, q, tcns=range(4)):
            if b not in x_sb:
                x_sb[b] = xpool.tile([128, 8, S], F16, tag="xb", name="x_sb")
            for i, tcn in enumerate(tcns):
                for kc in range(8):
                    q[(i * 8 + kc) % len(q)].dma_start(
                        out=x_sb[b][:, kc, tcn * 512:(tcn + 1) * 512],
                        in_=xT_d[kc * 128:(kc + 1) * 128,
                                 b * S + tcn * 512:b * S + (tcn + 1) * 512])

        # ---- building blocks ----
        def qk_chunk(kind, b, tcn):
            w_sb = wq_sb if kind == "q" else wk_sb
            t_sb = qT_sb if kind == "q" else kT_sb
            ps = shp.tile([128, 512], F32, tag="sh", name="ps")
            for kc in range(8):
                nc.tensor.matmul(ps[:], w_sb[:, kc, :],
                                 x_sb[b][:, kc, tcn * 512:(tcn + 1) * 512],
                                 start=kc == 0, stop=kc == 7)
            tsl = slice(b * S + tcn * 512, b * S + (tcn + 1) * 512)
            nc.vector.tensor_copy(t_sb[:, tsl], ps[:])

        def v_chunk(b, tcn):
            # natural-layout v; ONE accumulation group for the whole bank
            # (start=True clears has_written bank-wide for written partitions)
            ps = shp.tile([128, 4, 128], F32, tag="sh", name="vps")
            for kc in range(8):
                for sub in range(4):
                    nc.tensor.matmul(
                        ps[:, sub, :],
                        x_sb[b][:, kc, tcn * 512 + sub * 128:
                                tcn * 512 + (sub + 1) * 128],
                        wv_sb[:, kc, :],
                        start=(kc == 0 and sub == 0),
                        stop=(kc == 7 and sub == 3),
                        skip_group_check=True)
            base = b * 16 + tcn * 4
            d0 = v_all[:, base, 0:HD]
            dst = bass.AP(tensor=d0.tensor, offset=d0.offset,
                          ap=[list(d0.ap[0]), [HD, 4], [32 * HD, 2], [1, HD]])
            s0 = ps[:, 0, 0:HD]
            src = bass.AP(tensor=s0.tensor, offset=s0.offset,
                          ap=[list(s0.ap[0]), [128, 4], [HD, 2], [1, HD]])
            nc.vector.tensor_copy(dst, src)

        def rope_half(t_sb, b, half, rot_eng):
            # t[:, h] = t[:, h]*cos[h] + rot(t)[:, h]*sin[h] for one
            # 1024-col half of batch b.  rot = partition swap via DMA.
            hsl = slice(half * 1024, (half + 1) * 1024)
            bsl = slice(b * S + half * 1024, b * S + (half + 1) * 1024)
            rot = rotp.tile([128, 1024], F16, tag="rot", name="rot")
            nc.sync.dma_start(out=rot[0:32, :], in_=t_sb[32:64, bsl])
            nc.gpsimd.dma_start(out=rot[32:64, :], in_=t_sb[0:32, bsl])
            nc.sync.dma_start(out=rot[64:96, :], in_=t_sb[96:128, bsl])
            nc.gpsimd.dma_start(out=rot[96:128, :], in_=t_sb[64:96, bsl])
            rot_eng.tensor_mul(rot[:], rot[:], sin_sb[:, hsl])
            nc.vector.tensor_mul(t_sb[:, bsl], t_sb[:, bsl], cos_sb[:, hsl])
            nc.vector.tensor_add(t_sb[:, bsl], t_sb[:, bsl], rot[:])

        ot_cache = {}

        def proj_unit(b, qb, oc, pool=None, tag="sh", dmaq=nc.gpsimd):
            src = ctx0_sb if b == 0 else ctx1_sb
            qsl = slice(qb * 128, (qb + 1) * 128)
            osl = slice(oc * 512, (oc + 1) * 512)
            ps = (pool or shp).tile([128, 512], F32, tag=tag, name="pps")
            nc.tensor.matmul(ps[:], src[:, qsl], wo_sb[:, osl],
                             start=True, stop=True)
            if oc == 0:
                ot_cache[(b, qb)] = opool.tile([128, HID], F16, tag="ot",
                                               name="ot")
            ot = ot_cache[(b, qb)]
            nc.vector.tensor_copy(ot[:, osl], ps[:])
            if oc == 1:
                r0 = b * S + qb * 128
                dmaq.dma_start(out=out_d[r0:r0 + 128, :], in_=ot[:])
                del ot_cache[(b, qb)]

        # ---- deadline-tagged work queue ----
        # item: (cost_ns, deadline (qi, kb), emit_fn); FIFO order preserved.
        # Each drain() call adds `budget` of PE credit; items emit when
        # enough credit accumulated OR their deadline is due.
        work = []
        credit = [0]

        def drain(now, budget):
            credit[0] = min(credit[0] + budget, 6000)
            while work:
                cost, dl, fn = work[0]
                if dl > now and credit[0] < cost:
                    break
                work.pop(0)
                fn()
                credit[0] = max(credit[0] - cost, -6000)
        END = (9, 0)

        # ---- lead-in (batch 0: k + rope, q half 0, v chunk 0) ----
        with nc.named_scope("lead"):
            load_x(0, [nc.sync, nc.gpsimd, nc.scalar])
            for tcn in range(4):
                qk_chunk("k", 0, tcn)
            rope_half(kT_sb, 0, 0, nc.gpsimd)
            rope_half(kT_sb, 0, 1, nc.gpsimd)
            qk_chunk("q", 0, 0)
            qk_chunk("q", 0, 1)
            rope_half(qT_sb, 0, 0, nc.vector)
            v_chunk(0, 0)

        # ---- queue the rest, with deadlines ----
        A = work.append
        CH, PR, RP = 2600, 330, 1100
        A((CH, (0, 2), lambda: v_chunk(0, 1)))
        A((CH, (0, 6), lambda: v_chunk(0, 2)))
        A((CH, (0, 10), lambda: v_chunk(0, 3)))
        A((CH, (1, 0), lambda: qk_chunk("q", 0, 2)))
        A((CH, (1, 0), lambda: qk_chunk("q", 0, 3)))
        A((RP, (1, 0), lambda: rope_half(qT_sb, 0, 1, nc.vector)))
        A((200, (1, 0), lambda: load_x(1, [nc.sync, nc.gpsimd])))
        for t in range(4):
            A((CH, (2, 0), lambda t=t: qk_chunk("k", 1, t)))
        A((RP, (2, 0), lambda: rope_half(kT_sb, 1, 0, nc.gpsimd)))
        A((RP, (2, 0), lambda: rope_half(kT_sb, 1, 1, nc.gpsimd)))
        A((CH, (2, 0), lambda: qk_chunk("q", 1, 0)))
        A((CH, (2, 0), lambda: qk_chunk("q", 1, 1)))
        A((RP, (2, 0), lambda: rope_half(qT_sb, 1, 0, nc.vector)))
        A((CH, (2, 0), lambda: v_chunk(1, 0)))
        A((CH, (2, 2), lambda: v_chunk(1, 1)))
        A((CH, (2, 6), lambda: v_chunk(1, 2)))
        A((CH, (2, 10), lambda: v_chunk(1, 3)))
        A((CH, (3, 0), lambda: qk_chunk("q", 1, 2)))
        A((CH, (3, 0), lambda: qk_chunk("q", 1, 3)))
        A((RP, (3, 0), lambda: rope_half(qT_sb, 1, 1, nc.vector)))

        def queue_proj(b, qb_range):
            for qb in qb_range:
                for oc in range(2):
                    A((PR, END, lambda a=qb, o=oc, bb=b: proj_unit(bb, a, o)))

        # ---- attention quarters ----
        for qi, (b, qc) in enumerate([(0, 0), (0, 1), (1, 0), (1, 1)]):
            ctx_sb = ctx0_sb if b == 0 else ctx1_sb
            # previous quarters' proj becomes available now
            if qi == 1:
                queue_proj(0, range(8))
            elif qi == 2:
                queue_proj(0, range(8, 16))
            elif qi == 3:
                queue_proj(1, range(8))
            with nc.named_scope(f"attn{qi}"):
                q0 = b * S + qc * 1024
                csl = slice(qc * 1024, (qc + 1) * 1024)
                ctx_t = ctxp.tile([128, 1024], F32, tag="ctx", name="ctx_t")
                l_t = lpp.tile([128, 512], F32, tag="l", name="l_t")
                pring = {}
                for kb in range(18):
                    # deadlines must fire BEFORE this kb's attention work is
                    # emitted (emission order defines dependency direction)
                    drain((qi, kb), 700)
                    if kb < 16:
                        k0 = b * S + kb * 128
                        ksl = slice(k0, k0 + 128)
                        sp0 = spp.tile([128, 1024], F32, tag="sp0", name="sp0")
                        sp1 = spp.tile([128, 1024], F32, tag="sp1", name="sp1")
                        for half in range(2):
                            qsl = slice(q0 + half * 512, q0 + (half + 1) * 512)
                            hsl = slice(half * 512, (half + 1) * 512)
                            nc.tensor.matmul(sp0[:, hsl], kT_sb[0:64, ksl],
                                             qT_sb[0:64, qsl],
                                             start=True, stop=True)
                            nc.tensor.matmul(sp1[:, hsl], kT_sb[64:128, ksl],
                                             qT_sb[64:128, qsl],
                                             start=True, stop=True)
                        p0 = ppool.tile([128, 1024], F16, tag="p0", name="p0")
                        p1 = ppool.tile([128, 1024], F16, tag="p1", name="p1")
                        nc.scalar.activation(
                            out=p0[:], in_=sp0[:],
                            func=mybir.ActivationFunctionType.Exp)
                        nc.scalar.activation(
                            out=p1[:], in_=sp1[:],
                            func=mybir.ActivationFunctionType.Exp)
                        pring[kb] = (p0, p1)
                        if dbg is not None and qi == 0 and kb == 0:
                            dp = opool.tile([128, 1024], F32, tag="dbgp", bufs=1)
                            nc.vector.tensor_copy(dp[:], p0[:])
                            nc.sync.dma_start(out=dbg["p0"][:], in_=dp[:])
                    if kb >= 2:
                        kv = kb - 2
                        p0, p1 = pring.pop(kv)
                        sl0 = b * 16 + kv
                        sl1 = 32 + b * 16 + kv
                        st, sp_ = kv == 0, kv == 15
                        for half in range(2):
                            hsl = slice(half * 512, (half + 1) * 512)
                            nc.tensor.matmul(ctx_t[0:64, hsl],
                                             v_all[:, sl0, :], p0[:, hsl],
                                             start=st, stop=sp_)
                            nc.tensor.matmul(ctx_t[64:128, hsl],
                                             v_all[:, sl1, :], p1[:, hsl],
                                             start=st, stop=sp_)
                        nc.tensor.matmul(l_t[0:32, :], ones_sb[:],
                                         p0[:, 0:512], start=st, stop=sp_,
                                         tile_position=(0, 0))
                        nc.tensor.matmul(l_t[32:64, :], ones_sb[:],
                                         p0[:, 512:1024], start=st, stop=sp_,
                                         tile_position=(0, 32))
                        nc.tensor.matmul(l_t[64:96, :], ones_sb[:],
                                         p1[:, 0:512], start=st, stop=sp_,
                                         tile_position=(0, 64))
                        nc.tensor.matmul(l_t[96:128, :], ones_sb[:],
                                         p1[:, 512:1024], start=st, stop=sp_,
                                         tile_position=(0, 96))

                # ---- normalize this quarter ----
                cuh = lpool.tile([128, 1024], F32, tag="cu", name="cuh")
                nc.vector.tensor_copy(cuh[:], ctx_t[:])
                if dbg is not None and qi == 0:
                    nc.sync.dma_start(out=dbg["ctxq1"][:], in_=cuh[:])
                lsb = lpool.tile([128, 512], F32, tag="ls", name="lsb")
                nc.vector.tensor_copy(lsb[:], l_t[:])
                lrec = lpool.tile([128, 512], F32, tag="lr", name="lrec")
                nc.vector.reciprocal_approx_fast(out=lrec[:], in_=lsb[:])
                if dbg is not None and qi == 0:
                    nc.sync.dma_start(out=dbg["lq"][:], in_=lrec[:])
                if dbg is not None and qi == 1:
                    nc.sync.dma_start(out=dbg["ls2"][:], in_=lsb[:])
                    nc.sync.dma_start(out=dbg["lq2"][:], in_=lrec[:])
                nq = nc.scalar if qi == 3 else nc.sync
                nq.dma_start(out=rscr_d[qi, 0:97, :], in_=lrec[0:97, :])
                bct = bpool.tile([128, 1024], F32, tag="bct", name="bct")
                for h in range(2):
                    for half in range(2):
                        rr = rscr_d[qi, (h * 2 + half) * 32, :]
                        nq.dma_start(
                            out=bct[h * 64:(h + 1) * 64,
                                    half * 512:(half + 1) * 512],
                            in_=bass.AP(tensor=rr.tensor, offset=rr.offset,
                                        ap=[[0, 64], [1, 512]]))
                if dbg is not None and qi == 0:
                    nc.sync.dma_start(out=dbg["bct1"][:], in_=bct[:])
                nc.vector.tensor_mul(ctx_sb[0:64, csl], cuh[0:64, :],
                                     bct[0:64, :])
                nc.vector.tensor_mul(ctx_sb[64:128, csl], cuh[64:128, :],
                                     bct[64:128, :])

        # ---- tail: drain leftovers, then Q4's proj over 4 psum slots ----
        with nc.named_scope("tail"):
            drain(END, 10**9)
            units = [(1, qb, oc) for qb in range(8, 16) for oc in range(2)]
            pools = [(spp, "sp0"), (spp, "sp1"), (ctxp, "ctx"), (shp, "sh")]
            for i, (b_, qb, oc) in enumerate(units):
                pl, tg = pools[i % 4]
                proj_unit(b_, qb, oc, pool=pl, tag=tg,
                          dmaq=(nc.sync, nc.gpsimd, nc.scalar)[i % 3])

        if dbg is not None:
            nc.gpsimd.dma_start(out=dbg["qT"][:], in_=qT_sb[:])
            nc.gpsimd.dma_start(out=dbg["kT"][:], in_=kT_sb[:])
            dv = opool.tile([128, 64], F32, tag="dbgv", bufs=1)
            nc.vector.tensor_copy(dv[:], v_all[:, 0, :])
            nc.sync.dma_start(out=dbg["v0"][:], in_=dv[:])
            dva = opool.tile([128, 64 * HD], F32, tag="dbgva", bufs=1)
            nc.vector.tensor_copy(dva[:], v_all[:].rearrange("p a b -> p (a b)"))
            nc.sync.dma_start(out=dbg["vall"][:], in_=dva[:])
            dc = opool.tile([128, S], F32, tag="dbgc", bufs=1)
            nc.vector.tensor_copy(dc[:], ctx0_sb[:])
            nc.sync.dma_start(out=dbg["ctx0"][:], in_=dc[:])


def _swz(w):
    # [1024, 128] -> [128, 1024]: SBUF layout [p, kc*128+d] = w[kc*128+p, d]
    return np.ascontiguousarray(
        w.reshape(8, 128, 128).transpose(1, 0, 2).reshape(128, 1024))


def _prep_inputs(x, Wq, Wk, Wv, Wo):
    x2 = np.asarray(x, dtype=np.float32).reshape(T, HID)
    xT16 = np.ascontiguousarray(x2.T).astype(np.float16)

    half = HD // 2
    inv_freq = (1.0 / (ROPE_BASE ** (np.arange(half, dtype=np.float64) * 2.0 / HD)))
    ang = np.arange(S, dtype=np.float64)[None, :] * inv_freq[:, None]  # [32, S]
    cosf = np.tile(np.cos(ang), (4, 1)).astype(np.float16)
    sgn = np.repeat([-1.0, 1.0, -1.0, 1.0], 32)[:, None]
    sins = (np.tile(np.sin(ang), (4, 1)) * sgn).astype(np.float16)

    scale = np.float32(1.0 / np.sqrt(HD))
    in_maps = []
    for c in range(NCORES):
        rows = slice(c * DPC, (c + 1) * DPC)
        in_maps.append({
            "xT16": xT16,
            "wq": _swz((Wq[rows, :] * scale).T.astype(np.float16)),
            "wk": _swz(Wk[rows, :].T.astype(np.float16)),
            "wv": _swz(Wv[rows, :].T.astype(np.float16)),
            "wo": np.ascontiguousarray(Wo[:, rows].T).astype(np.float16),
            "cosf": cosf,
            "sins": sins,
        })
    return in_maps


def _run(in_maps, trace=False):
    if "nc" not in _CACHE:
        _CACHE["nc"] = _build_program()
    nc = _CACHE["nc"]
    res = run_bass_kernel_spmd(nc, in_maps, core_ids=list(range(NCORES)),
                               trace=trace)
    acc = res.results[0]["out"].astype(np.float32).copy()
    for c in range(1, NCORES):
        acc += res.results[c]["out"].astype(np.float32)
    return acc.reshape(B, S, HID), res


def kernel(x, Wq, Wk, Wv, Wo):
    in_maps = _prep_inputs(np.asarray(x), np.asarray(Wq), np.asarray(Wk),
                           np.asarray(Wv), np.asarray(Wo))
    out, _ = _run(in_maps, trace=False)
    return out


def run_profiled(x, Wq, Wk, Wv, Wo):
    in_maps = _prep_inputs(np.asarray(x), np.asarray(Wq), np.asarray(Wk),
                           np.asarray(Wv), np.asarray(Wo))
    return _run(in_maps, trace=True)


# revision 19
# speedup vs baseline: 1.0487x; 1.0487x over previous
"""Multi-head attention with RoPE (B=2, S=2048, H=16, D=64) on 8 TRN2 cores,
tensor-parallel over heads (2 heads/core); host sums the 8 rank-128 fp16
partial outputs.

Restructured from the 252us baseline around two engine budgets:
  - ScalarE runs ONLY the 128 softmax exps ([128,1024] each, ~1.1us);
    everything else (copies, DMA issue) lives on DVE/Sync/GpSimd queues.
  - PE work cut via tile_position packing:
      * scores: h0 (rows 0-63) and h1 (rows 64-127) issue as adjacent
        K=64 matmuls on disjoint row-groups -> run concurrently.
      * PV: v has no ones column (M=64); h0 writes ctx psum partitions
        0-63, h1 partitions 64-127 (col tile (0,64)), concurrently from
        the two p streams.  start=True clears has_written for the whole
        bank but only on the written partitions, so partition-disjoint
        groups are safe (free-offset-disjoint groups are NOT - see
        v_chunk, which uses a single accumulation group per bank).
      * softmax denominators via an "l-quad": four M=1 ones-stationary
        matmuls col-tiled at psum partitions 0/32/64/96, accumulating
        sum_k exp over the kb loop.
  - Single fused timeline: lead-in computes k(b0), rope-k, q(b0) first
    half, rope-q half0, v(b0) chunk0; everything else (rest of b0,
    qkv(b1), output projection) drains from a deadline-tagged work queue
    inside the attention kb loops, paced to fill PE slack under the
    ScalarE exp cadence.
  - PSUM (8 banks): sp0 2 + sp1 2 + packed ctx 2 + l 1 + one shared slot
    (qkv-psum / v-psum / proj) 1.
  - x cached in SBUF per batch (1 HBM read), loaded in [128,512] pieces
    token-major so the first k-projection starts after ~1 piece.
  - outputs stored fp16 (host sums in fp32).
"""
import numpy as np
import ml_dtypes

import concourse.bass as bass
import concourse.mybir as mybir
import concourse.tile as tile
from concourse import bacc
from concourse.bass_utils import run_bass_kernel_spmd

F32 = mybir.dt.float32
F16 = mybir.dt.float16

B, S, HID = 2, 2048, 1024
NH, HD = 16, 64
T = B * S
NCORES = 8
HPC = NH // NCORES         # 2 heads per core
DPC = HPC * HD             # 128 context dims per core
ROPE_BASE = 10000.0

_CACHE = {}
DEBUG_TAPS = False


def _build_program():
    nc = bacc.Bacc("TRN2", target_bir_lowering=False, debug=False)

    xT_d = nc.dram_tensor("xT16", [HID, T], F16, kind="ExternalInput")
    wq_d = nc.dram_tensor("wq", [128, HID], F16, kind="ExternalInput")
    wk_d = nc.dram_tensor("wk", [128, HID], F16, kind="ExternalInput")
    wv_d = nc.dram_tensor("wv", [128, HID], F16, kind="ExternalInput")
    wo_d = nc.dram_tensor("wo", [DPC, HID], F16, kind="ExternalInput")
    cos_d = nc.dram_tensor("cosf", [128, S], F16, kind="ExternalInput")
    sin_d = nc.dram_tensor("sins", [128, S], F16, kind="ExternalInput")
    out_d = nc.dram_tensor("out", [T, HID], F16, kind="ExternalOutput")
    rscr_d = nc.dram_tensor("rscr", [4, 128, 512], F32,
                            kind="ExternalOutput" if DEBUG_TAPS else "Internal")

    dbg = None
    if DEBUG_TAPS:
        dbg = {
            "qT": nc.dram_tensor("dbg_qT", [128, T], F32, kind="ExternalOutput"),
            "kT": nc.dram_tensor("dbg_kT", [128, T], F32, kind="ExternalOutput"),
            "v0": nc.dram_tensor("dbg_v0", [128, 64], F32, kind="ExternalOutput"),
            "p0": nc.dram_tensor("dbg_p0", [128, 1024], F32, kind="ExternalOutput"),
            "lq": nc.dram_tensor("dbg_lq", [128, 512], F32, kind="ExternalOutput"),
            "ls2": nc.dram_tensor("dbg_ls2", [128, 512], F32, kind="ExternalOutput"),
            "lq2": nc.dram_tensor("dbg_lq2", [128, 512], F32, kind="ExternalOutput"),
            "ctx0": nc.dram_tensor("dbg_ctx0", [128, S], F32, kind="ExternalOutput"),
            "vall": nc.dram_tensor("dbg_vall", [128, 64 * HD], F32, kind="ExternalOutput"),
            "ctxq1": nc.dram_tensor("dbg_ctxq1", [128, 1024], F32, kind="ExternalOutput"),
            "bct1": nc.dram_tensor("dbg_bct1", [128, 1024], F32, kind="ExternalOutput"),
        }

    with tile.TileContext(nc) as tc:
        _emit(nc, tc, xT_d, wq_d, wk_d, wv_d, wo_d, cos_d, sin_d, out_d,
              rscr_d, dbg=dbg)
    nc.compile()
    return nc


def _emit(nc, tc, xT_d, wq_d, wk_d, wv_d, wo_d, cos_d, sin_d, out_d,
          rscr_d, dbg=None):
    import contextlib
    ctx = contextlib.ExitStack()
    with ctx:
        singles = ctx.enter_context(tc.tile_pool(name="singles", bufs=1))
        xpool = ctx.enter_context(tc.tile_pool(name="xpool", bufs=2))
        ppool = ctx.enter_context(tc.tile_pool(name="ppool", bufs=3))
        rotp = ctx.enter_context(tc.tile_pool(name="rotp", bufs=3))
        lpool = ctx.enter_context(tc.tile_pool(name="lpool", bufs=2))
        bpool = ctx.enter_context(tc.tile_pool(name="bpool", bufs=2))
        opool = ctx.enter_context(tc.tile_pool(name="opool", bufs=3))
        # PSUM: exactly 8 banks
        spp = ctx.enter_context(tc.tile_pool(name="spp", bufs=1, space="PSUM"))
        ctxp = ctx.enter_context(tc.tile_pool(name="ctxp", bufs=1, space="PSUM"))
        lpp = ctx.enter_context(tc.tile_pool(name="lpp", bufs=1, space="PSUM"))
        shp = ctx.enter_context(tc.tile_pool(name="shp", bufs=1, space="PSUM"))

        # ---- persistent SBUF ----
        wq_sb = singles.tile([128, 8, DPC], F16)
        wk_sb = singles.tile([128, 8, DPC], F16)
        wv_sb = singles.tile([128, 8, DPC], F16)
        wo_sb = singles.tile([128, HID], F16)
        cos_sb = singles.tile([128, S], F16)
        sin_sb = singles.tile([128, S], F16)
        qT_sb = singles.tile([128, T], F16)
        kT_sb = singles.tile([128, T], F16)
        v_all = singles.tile([128, 64, HD], F16)   # slot = h*32 + b*16 + kb
        ctx0_sb = singles.tile([128, S], F16)      # normalized ctx^T, b=0
        ctx1_sb = singles.tile([128, S], F16)
        ones_sb = singles.tile([128, 32], F16)

        nc.sync.dma_start(out=wq_sb[:].rearrange("p a b -> p (a b)"), in_=wq_d[:])
        nc.scalar.dma_start(out=wk_sb[:].rearrange("p a b -> p (a b)"), in_=wk_d[:])
        nc.gpsimd.dma_start(out=wv_sb[:].rearrange("p a b -> p (a b)"), in_=wv_d[:])
        nc.gpsimd.dma_start(out=wo_sb[:], in_=wo_d[:])
        nc.scalar.dma_start(out=cos_sb[:], in_=cos_d[:])
        nc.scalar.dma_start(out=sin_sb[:], in_=sin_d[:])
        nc.vector.memset(ones_sb[:], 1.0)
        # preload the exp table set during the lead-in
        tblw = singles.tile([1, 8], F32)
        nc.vector.memset(tblw[:], 0.0)
        nc.scalar.activation(out=tblw[:], in_=tblw[:],
                             func=mybir.ActivationFunctionType.Exp)

        # x cache: one SBUF slot per batch, [128, kc, tokens-in-batch],
        # loaded token-major in [128,512] pieces so consumers start early
        x_sb = {}

        def load_x(b, q, tcns=range(4)):
            if b not in x_sb:
                x_sb[b] = xpool.tile([128, 8, S], F16, tag="xb", name="x_sb")
            for i, tcn in enumerate(tcns):
                for kc in range(8):
                    q[(i * 8 + kc) % len(q)].dma_start(
                        out=x_sb[b][:, kc, tcn * 512:(tcn + 1) * 512],
                        in_=xT_d[kc * 128:(kc + 1) * 128,
                                 b * S + tcn * 512:b * S + (tcn + 1) * 512])

        # ---- building blocks ----
        def qk_chunk(kind, b, tcn):
            w_sb = wq_sb if kind == "q" else wk_sb
            t_sb = qT_sb if kind == "q" else kT_sb
            ps = shp.tile([128, 512], F32, tag="sh", name="ps")
            for kc in range(8):
                nc.tensor.matmul(ps[:], w_sb[:, kc, :],
                                 x_sb[b][:, kc, tcn * 512:(tcn + 1) * 512],
                                 start=kc == 0, stop=kc == 7)
            tsl = slice(b * S + tcn * 512, b * S + (tcn + 1) * 512)
            nc.vector.tensor_copy(t_sb[:, tsl], ps[:])

        def v_chunk(b, tcn):
            # natural-layout v; ONE accumulation group for the whole bank
            # (start=True clears has_written bank-wide for written partitions)
            ps = shp.tile([128, 4, 128], F32, tag="sh", name="vps")
            for kc in range(8):
                for sub in range(4):
                    nc.tensor.matmul(
                        ps[:, sub, :],
                        x_sb[b][:, kc, tcn * 512 + sub * 128:
                                tcn * 512 + (sub + 1) * 128],
                        wv_sb[:, kc, :],
                        start=(kc == 0 and sub == 0),
                        stop=(kc == 7 and sub == 3),
                        skip_group_check=True)
            base = b * 16 + tcn * 4
            d0 = v_all[:, base, 0:HD]
            dst = bass.AP(tensor=d0.tensor, offset=d0.offset,
                          ap=[list(d0.ap[0]), [HD, 4], [32 * HD, 2], [1, HD]])
            s0 = ps[:, 0, 0:HD]
            src = bass.AP(tensor=s0.tensor, offset=s0.offset,
                          ap=[list(s0.ap[0]), [128, 4], [HD, 2], [1, HD]])
            nc.vector.tensor_copy(dst, src)

        def rope_half(t_sb, b, half, rot_eng):
            # t[:, h] = t[:, h]*cos[h] + rot(t)[:, h]*sin[h] for one
            # 1024-col half of batch b.  rot = partition swap via DMA.
            hsl = slice(half * 1024, (half + 1) * 1024)
            bsl = slice(b * S + half * 1024, b * S + (half + 1) * 1024)
            rot = rotp.tile([128, 1024], F16, tag="rot", name="rot")
            nc.sync.dma_start(out=rot[0:32, :], in_=t_sb[32:64, bsl])
            nc.gpsimd.dma_start(out=rot[32:64, :], in_=t_sb[0:32, bsl])
            nc.sync.dma_start(out=rot[64:96, :], in_=t_sb[96:128, bsl])
            nc.gpsimd.dma_start(out=rot[96:128, :], in_=t_sb[64:96, bsl])
            rot_eng.tensor_mul(rot[:], rot[:], sin_sb[:, hsl])
            nc.vector.tensor_mul(t_sb[:, bsl], t_sb[:, bsl], cos_sb[:, hsl])
            nc.vector.tensor_add(t_sb[:, bsl], t_sb[:, bsl], rot[:])

        ot_cache = {}

        def proj_unit(b, qb, oc, pool=None, tag="sh", dmaq=nc.gpsimd):
            src = ctx0_sb if b == 0 else ctx1_sb
            qsl = slice(qb * 128, (qb + 1) * 128)
            osl = slice(oc * 512, (oc + 1) * 512)
            ps = (pool or shp).tile([128, 512], F32, tag=tag, name="pps")
            nc.tensor.matmul(ps[:], src[:, qsl], wo_sb[:, osl],
                             start=True, stop=True)
            if oc == 0:
                ot_cache[(b, qb)] = opool.tile([128, HID], F16, tag="ot",
                                               name="ot")
            ot = ot_cache[(b, qb)]
            nc.vector.tensor_copy(ot[:, osl], ps[:])
            if oc == 1:
                r0 = b * S + qb * 128
                dmaq.dma_start(out=out_d[r0:r0 + 128, :], in_=ot[:])
                del ot_cache[(b, qb)]

        # ---- deadline-tagged work queue ----
        # item: (cost_ns, deadline (qi, kb), emit_fn); FIFO order preserved.
        # Each drain() call adds `budget` of PE credit; items emit when
        # enough credit accumulated OR their deadline is due.
        work = []
        credit = [0]

        def drain(now, budget):
            credit[0] = min(credit[0] + budget, 6000)
            while work:
                cost, dl, fn = work[0]
                if dl > now and credit[0] < cost:
                    break
                work.pop(0)
                fn()
                credit[0] = max(credit[0] - cost, -6000)
        END = (9, 0)

        # ---- lead-in: ALL of batch 0's qkv + rope (PE-dense, DMA-paced) ----
        with nc.named_scope("lead"):
            load_x(0, [nc.sync, nc.gpsimd, nc.scalar])
            for tcn in range(4):
                qk_chunk("k", 0, tcn)
            rope_half(kT_sb, 0, 0, nc.gpsimd)
            rope_half(kT_sb, 0, 1, nc.vector)
            qk_chunk("q", 0, 0)
            qk_chunk("q", 0, 1)
            rope_half(qT_sb, 0, 0, nc.gpsimd)
            v_chunk(0, 0)
            qk_chunk("q", 0, 2)
            qk_chunk("q", 0, 3)
            rope_half(qT_sb, 0, 1, nc.vector)
            v_chunk(0, 1)
            v_chunk(0, 2)
            v_chunk(0, 3)

        # ---- queue batch 1's qkv + all proj, with deadlines ----
        A = work.append
        CH, PR, RP = 2600, 330, 1100
        A((200, (0, 2), lambda: load_x(1, [nc.sync, nc.gpsimd])))
        for t in range(4):
            A((CH, (2, 0), lambda t=t: qk_chunk("k", 1, t)))
        A((RP, (2, 0), lambda: rope_half(kT_sb, 1, 0, nc.gpsimd)))
        A((RP, (2, 0), lambda: rope_half(kT_sb, 1, 1, nc.gpsimd)))
        A((CH, (2, 0), lambda: qk_chunk("q", 1, 0)))
        A((CH, (2, 0), lambda: qk_chunk("q", 1, 1)))
        A((RP, (2, 0), lambda: rope_half(qT_sb, 1, 0, nc.vector)))
        A((CH, (2, 0), lambda: v_chunk(1, 0)))
        A((CH, (2, 2), lambda: v_chunk(1, 1)))
        A((CH, (2, 6), lambda: v_chunk(1, 2)))
        A((CH, (2, 10), lambda: v_chunk(1, 3)))
        A((CH, (3, 0), lambda: qk_chunk("q", 1, 2)))
        A((CH, (3, 0), lambda: qk_chunk("q", 1, 3)))
        A((RP, (3, 0), lambda: rope_half(qT_sb, 1, 1, nc.vector)))

        def queue_proj(b, qb_range):
            for qb in qb_range:
                for oc in range(2):
                    A((PR, END, lambda a=qb, o=oc, bb=b: proj_unit(bb, a, o)))

        # ---- attention quarters ----
        for qi, (b, qc) in enumerate([(0, 0), (0, 1), (1, 0), (1, 1)]):
            ctx_sb = ctx0_sb if b == 0 else ctx1_sb
            # previous quarters' proj becomes available now (Q1's deferred
            # to Q3 so Q2's slack stays for b1's q projections)
            if qi == 2:
                queue_proj(0, range(8))
                queue_proj(0, range(8, 16))
            elif qi == 3:
                queue_proj(1, range(8))
            with nc.named_scope(f"attn{qi}"):
                q0 = b * S + qc * 1024
                csl = slice(qc * 1024, (qc + 1) * 1024)
                ctx_t = ctxp.tile([128, 1024], F32, tag="ctx", name="ctx_t")
                l_t = lpp.tile([128, 512], F32, tag="l", name="l_t")
                pring = {}
                for kb in range(18):
                    # deadlines must fire BEFORE this kb's attention work is
                    # emitted (emission order defines dependency direction)
                    drain((qi, kb), 700)
                    if kb < 16:
                        k0 = b * S + kb * 128
                        ksl = slice(k0, k0 + 128)
                        sp0 = spp.tile([128, 1024], F32, tag="sp0", name="sp0")
                        sp1 = spp.tile([128, 1024], F32, tag="sp1", name="sp1")
                        for half in range(2):
                            qsl = slice(q0 + half * 512, q0 + (half + 1) * 512)
                            hsl = slice(half * 512, (half + 1) * 512)
                            nc.tensor.matmul(sp0[:, hsl], kT_sb[0:64, ksl],
                                             qT_sb[0:64, qsl],
                                             start=True, stop=True)
                            nc.tensor.matmul(sp1[:, hsl], kT_sb[64:128, ksl],
                                             qT_sb[64:128, qsl],
                                             start=True, stop=True)
                        p0 = ppool.tile([128, 1024], F16, tag="p0", name="p0")
                        p1 = ppool.tile([128, 1024], F16, tag="p1", name="p1")
                        nc.scalar.activation(
                            out=p0[:], in_=sp0[:],
                            func=mybir.ActivationFunctionType.Exp)
                        nc.scalar.activation(
                            out=p1[:], in_=sp1[:],
                            func=mybir.ActivationFunctionType.Exp)
                        pring[kb] = (p0, p1)
                        if dbg is not None and qi == 0 and kb == 0:
                            dp = opool.tile([128, 1024], F32, tag="dbgp", bufs=1)
                            nc.vector.tensor_copy(dp[:], p0[:])
                            nc.sync.dma_start(out=dbg["p0"][:], in_=dp[:])
                    if kb >= 2:
                        kv = kb - 2
                        p0, p1 = pring.pop(kv)
                        sl0 = b * 16 + kv
                        sl1 = 32 + b * 16 + kv
                        st, sp_ = kv == 0, kv == 15
                        for half in range(2):
                            hsl = slice(half * 512, (half + 1) * 512)
                            nc.tensor.matmul(ctx_t[0:64, hsl],
                                             v_all[:, sl0, :], p0[:, hsl],
                                             start=st, stop=sp_)
                            nc.tensor.matmul(ctx_t[64:128, hsl],
                                             v_all[:, sl1, :], p1[:, hsl],
                                             start=st, stop=sp_)
                        nc.tensor.matmul(l_t[0:32, :], ones_sb[:],
                                         p0[:, 0:512], start=st, stop=sp_,
                                         tile_position=(0, 0))
                        nc.tensor.matmul(l_t[32:64, :], ones_sb[:],
                                         p0[:, 512:1024], start=st, stop=sp_,
                                         tile_position=(0, 32))
                        nc.tensor.matmul(l_t[64:96, :], ones_sb[:],
                                         p1[:, 0:512], start=st, stop=sp_,
                                         tile_position=(0, 64))
                        nc.tensor.matmul(l_t[96:128, :], ones_sb[:],
                                         p1[:, 512:1024], start=st, stop=sp_,
                                         tile_position=(0, 96))

                # ---- normalize this quarter ----
                cuh = lpool.tile([128, 1024], F32, tag="cu", name="cuh")
                nc.vector.tensor_copy(cuh[:], ctx_t[:])
                if dbg is not None and qi == 0:
                    nc.sync.dma_start(out=dbg["ctxq1"][:], in_=cuh[:])
                lsb = lpool.tile([128, 512], F32, tag="ls", name="lsb")
                nc.vector.tensor_copy(lsb[:], l_t[:])
                lrec = lpool.tile([128, 512], F32, tag="lr", name="lrec")
                nc.vector.reciprocal_approx_fast(out=lrec[:], in_=lsb[:])
                if dbg is not None and qi == 0:
                    nc.sync.dma_start(out=dbg["lq"][:], in_=lrec[:])
                if dbg is not None and qi == 1:
                    nc.sync.dma_start(out=dbg["ls2"][:], in_=lsb[:])
                    nc.sync.dma_start(out=dbg["lq2"][:], in_=lrec[:])
                nq = nc.scalar if qi == 3 else nc.sync
                nq.dma_start(out=rscr_d[qi, 0:97, :], in_=lrec[0:97, :])
                bct = bpool.tile([128, 1024], F32, tag="bct", name="bct")
                for h in range(2):
                    for half in range(2):
                        rr = rscr_d[qi, (h * 2 + half) * 32, :]
                        nq.dma_start(
                            out=bct[h * 64:(h + 1) * 64,
                                    half * 512:(half + 1) * 512],
                            in_=bass.AP(tensor=rr.tensor, offset=rr.offset,
                                        ap=[[0, 64], [1, 512]]))
                if dbg is not None and qi == 0:
                    nc.sync.dma_start(out=dbg["bct1"][:], in_=bct[:])
                nc.vector.tensor_mul(ctx_sb[0:64, csl], cuh[0:64, :],
                                     bct[0:64, :])
                nc.vector.tensor_mul(ctx_sb[64:128, csl], cuh[64:128, :],
                                     bct[64:128, :])

        # ---- tail: drain leftovers, then Q4's proj over 4 psum slots ----
        with nc.named_scope("tail"):
            drain(END, 10**9)
            units = [(1, qb, oc) for qb in range(8, 16) for oc in range(2)]
            pools = [(spp, "sp0"), (spp, "sp1"), (ctxp, "ctx"), (shp, "sh")]
            for i, (b_, qb, oc) in enumerate(units):
                pl, tg = pools[i % 4]
                proj_unit(b_, qb, oc, pool=pl, tag=tg,
                          dmaq=(nc.sync, nc.gpsimd, nc.scalar)[i % 3])

        if dbg is not None:
            nc.gpsimd.dma_start(out=dbg["qT"][:], in_=qT_sb[:])
            nc.gpsimd.dma_start(out=dbg["kT"][:], in_=kT_sb[:])
            dv = opool.tile([128, 64], F32, tag="dbgv", bufs=1)
            nc.vector.tensor_copy(dv[:], v_all[:, 0, :])
            nc.sync.dma_start(out=dbg["v0"][:], in_=dv[:])
            dva = opool.tile([128, 64 * HD], F32, tag="dbgva", bufs=1)
            nc.vector.tensor_copy(dva[:], v_all[:].rearrange("p a b -> p (a b)"))
            nc.sync.dma_start(out=dbg["vall"][:], in_=dva[:])
            dc = opool.tile([128, S], F32, tag="dbgc", bufs=1)
            nc.vector.tensor_copy(dc[:], ctx0_sb[:])
            nc.sync.dma_start(out=dbg["ctx0"][:], in_=dc[:])


def _swz(w):
    # [1024, 128] -> [128, 1024]: SBUF layout [p, kc*128+d] = w[kc*128+p, d]
    return np.ascontiguousarray(
        w.reshape(8, 128, 128).transpose(1, 0, 2).reshape(128, 1024))


def _prep_inputs(x, Wq, Wk, Wv, Wo):
    x2 = np.asarray(x, dtype=np.float32).reshape(T, HID)
    xT16 = np.ascontiguousarray(x2.T).astype(np.float16)

    half = HD // 2
    inv_freq = (1.0 / (ROPE_BASE ** (np.arange(half, dtype=np.float64) * 2.0 / HD)))
    ang = np.arange(S, dtype=np.float64)[None, :] * inv_freq[:, None]  # [32, S]
    cosf = np.tile(np.cos(ang), (4, 1)).astype(np.float16)
    sgn = np.repeat([-1.0, 1.0, -1.0, 1.0], 32)[:, None]
    sins = (np.tile(np.sin(ang), (4, 1)) * sgn).astype(np.float16)

    scale = np.float32(1.0 / np.sqrt(HD))
    in_maps = []
    for c in range(NCORES):
        rows = slice(c * DPC, (c + 1) * DPC)
        in_maps.append({
            "xT16": xT16,
            "wq": _swz((Wq[rows, :] * scale).T.astype(np.float16)),
            "wk": _swz(Wk[rows, :].T.astype(np.float16)),
            "wv": _swz(Wv[rows, :].T.astype(np.float16)),
            "wo": np.ascontiguousarray(Wo[:, rows].T).astype(np.float16),
            "cosf": cosf,
            "sins": sins,
        })
    return in_maps


def _run(in_maps, trace=False):
    if "nc" not in _CACHE:
        _CACHE["nc"] = _build_program()
    nc = _CACHE["nc"]
    res = run_bass_kernel_spmd(nc, in_maps, core_ids=list(range(NCORES)),
                               trace=trace)
    acc = res.results[0]["out"].astype(np.float32).copy()
    for c in range(1, NCORES):
        acc += res.results[c]["out"].astype(np.float32)
    return acc.reshape(B, S, HID), res


def kernel(x, Wq, Wk, Wv, Wo):
    in_maps = _prep_inputs(np.asarray(x), np.asarray(Wq), np.asarray(Wk),
                           np.asarray(Wv), np.asarray(Wo))
    out, _ = _run(in_maps, trace=False)
    return out


def run_profiled(x, Wq, Wk, Wv, Wo):
    in_maps = _prep_inputs(np.asarray(x), np.asarray(Wq), np.asarray(Wk),
                           np.asarray(Wv), np.asarray(Wo))
    return _run(in_maps, trace=True)


# revision 20
# speedup vs baseline: 1.0552x; 1.0061x over previous
"""Multi-head attention with RoPE (B=2, S=2048, H=16, D=64) on 8 TRN2 cores,
tensor-parallel over heads (2 heads/core); host sums the 8 rank-128 fp16
partial outputs.

Restructured from the 252us baseline around two engine budgets:
  - ScalarE runs ONLY the 128 softmax exps ([128,1024] each, ~1.1us);
    everything else (copies, DMA issue) lives on DVE/Sync/GpSimd queues.
  - PE work cut via tile_position packing:
      * scores: h0 (rows 0-63) and h1 (rows 64-127) issue as adjacent
        K=64 matmuls on disjoint row-groups -> run concurrently.
      * PV: v has no ones column (M=64); h0 writes ctx psum partitions
        0-63, h1 partitions 64-127 (col tile (0,64)), concurrently from
        the two p streams.  start=True clears has_written for the whole
        bank but only on the written partitions, so partition-disjoint
        groups are safe (free-offset-disjoint groups are NOT - see
        v_chunk, which uses a single accumulation group per bank).
      * softmax denominators via an "l-quad": four M=1 ones-stationary
        matmuls col-tiled at psum partitions 0/32/64/96, accumulating
        sum_k exp over the kb loop.
  - Single fused timeline: lead-in computes k(b0), rope-k, q(b0) first
    half, rope-q half0, v(b0) chunk0; everything else (rest of b0,
    qkv(b1), output projection) drains from a deadline-tagged work queue
    inside the attention kb loops, paced to fill PE slack under the
    ScalarE exp cadence.
  - PSUM (8 banks): sp0 2 + sp1 2 + packed ctx 2 + l 1 + one shared slot
    (qkv-psum / v-psum / proj) 1.
  - x cached in SBUF per batch (1 HBM read), loaded in [128,512] pieces
    token-major so the first k-projection starts after ~1 piece.
  - outputs stored fp16 (host sums in fp32).
"""
import numpy as np
import ml_dtypes

import concourse.bass as bass
import concourse.mybir as mybir
import concourse.tile as tile
from concourse import bacc
from concourse.bass_utils import run_bass_kernel_spmd

F32 = mybir.dt.float32
F16 = mybir.dt.float16

B, S, HID = 2, 2048, 1024
NH, HD = 16, 64
T = B * S
NCORES = 8
HPC = NH // NCORES         # 2 heads per core
DPC = HPC * HD             # 128 context dims per core
ROPE_BASE = 10000.0

_CACHE = {}
DEBUG_TAPS = False


def _build_program():
    nc = bacc.Bacc("TRN2", target_bir_lowering=False, debug=False)

    xT_d = nc.dram_tensor("xT16", [HID, T], F16, kind="ExternalInput")
    wq_d = nc.dram_tensor("wq", [128, HID], F16, kind="ExternalInput")
    wk_d = nc.dram_tensor("wk", [128, HID], F16, kind="ExternalInput")
    wv_d = nc.dram_tensor("wv", [128, HID], F16, kind="ExternalInput")
    wo_d = nc.dram_tensor("wo", [DPC, HID], F16, kind="ExternalInput")
    cos_d = nc.dram_tensor("cosf", [128, S], F16, kind="ExternalInput")
    sin_d = nc.dram_tensor("sins", [128, S], F16, kind="ExternalInput")
    out_d = nc.dram_tensor("out", [T, HID], F16, kind="ExternalOutput")
    rscr_d = nc.dram_tensor("rscr", [4, 128, 512], F32,
                            kind="ExternalOutput" if DEBUG_TAPS else "Internal")

    dbg = None
    if DEBUG_TAPS:
        dbg = {
            "qT": nc.dram_tensor("dbg_qT", [128, T], F32, kind="ExternalOutput"),
            "kT": nc.dram_tensor("dbg_kT", [128, T], F32, kind="ExternalOutput"),
            "v0": nc.dram_tensor("dbg_v0", [128, 64], F32, kind="ExternalOutput"),
            "p0": nc.dram_tensor("dbg_p0", [128, 1024], F32, kind="ExternalOutput"),
            "lq": nc.dram_tensor("dbg_lq", [128, 512], F32, kind="ExternalOutput"),
            "ls2": nc.dram_tensor("dbg_ls2", [128, 512], F32, kind="ExternalOutput"),
            "lq2": nc.dram_tensor("dbg_lq2", [128, 512], F32, kind="ExternalOutput"),
            "ctx0": nc.dram_tensor("dbg_ctx0", [128, S], F32, kind="ExternalOutput"),
            "vall": nc.dram_tensor("dbg_vall", [128, 64 * HD], F32, kind="ExternalOutput"),
            "ctxq1": nc.dram_tensor("dbg_ctxq1", [128, 1024], F32, kind="ExternalOutput"),
            "bct1": nc.dram_tensor("dbg_bct1", [128, 1024], F32, kind="ExternalOutput"),
        }

    with tile.TileContext(nc) as tc:
        _emit(nc, tc, xT_d, wq_d, wk_d, wv_d, wo_d, cos_d, sin_d, out_d,
              rscr_d, dbg=dbg)
    nc.compile()
    return nc


def _emit(nc, tc, xT_d, wq_d, wk_d, wv_d, wo_d, cos_d, sin_d, out_d,
          rscr_d, dbg=None):
    import contextlib
    ctx = contextlib.ExitStack()
    with ctx:
        singles = ctx.enter_context(tc.tile_pool(name="singles", bufs=1))
        xpool = ctx.enter_context(tc.tile_pool(name="xpool", bufs=2))
        ppool = ctx.enter_context(tc.tile_pool(name="ppool", bufs=3))
        rotp = ctx.enter_context(tc.tile_pool(name="rotp", bufs=3))
        lpool = ctx.enter_context(tc.tile_pool(name="lpool", bufs=2))
        bpool = ctx.enter_context(tc.tile_pool(name="bpool", bufs=2))
        opool = ctx.enter_context(tc.tile_pool(name="opool", bufs=3))
        # PSUM: exactly 8 banks
        spp = ctx.enter_context(tc.tile_pool(name="spp", bufs=1, space="PSUM"))
        ctxp = ctx.enter_context(tc.tile_pool(name="ctxp", bufs=1, space="PSUM"))
        lpp = ctx.enter_context(tc.tile_pool(name="lpp", bufs=1, space="PSUM"))
        shp = ctx.enter_context(tc.tile_pool(name="shp", bufs=1, space="PSUM"))

        # ---- persistent SBUF ----
        wq_sb = singles.tile([128, 8, DPC], F16)
        wk_sb = singles.tile([128, 8, DPC], F16)
        wv_sb = singles.tile([128, 8, DPC], F16)
        wo_sb = singles.tile([128, HID], F16)
        cos_sb = singles.tile([128, S], F16)
        sin_sb = singles.tile([128, S], F16)
        qT_sb = singles.tile([128, T], F16)
        kT_sb = singles.tile([128, T], F16)
        v_all = singles.tile([128, 64, HD], F16)   # slot = h*32 + b*16 + kb
        ctx0_sb = singles.tile([128, S], F16)      # normalized ctx^T, b=0
        ctx1_sb = singles.tile([128, S], F16)
        ones_sb = singles.tile([128, 32], F16)

        nc.sync.dma_start(out=wq_sb[:].rearrange("p a b -> p (a b)"), in_=wq_d[:])
        nc.scalar.dma_start(out=wk_sb[:].rearrange("p a b -> p (a b)"), in_=wk_d[:])
        nc.gpsimd.dma_start(out=wv_sb[:].rearrange("p a b -> p (a b)"), in_=wv_d[:])
        nc.gpsimd.dma_start(out=wo_sb[:], in_=wo_d[:])
        nc.scalar.dma_start(out=cos_sb[:], in_=cos_d[:])
        nc.scalar.dma_start(out=sin_sb[:], in_=sin_d[:])
        nc.vector.memset(ones_sb[:], 1.0)
        # preload the exp table set during the lead-in
        tblw = singles.tile([1, 8], F32)
        nc.vector.memset(tblw[:], 0.0)
        nc.scalar.activation(out=tblw[:], in_=tblw[:],
                             func=mybir.ActivationFunctionType.Exp)

        # x cache: one SBUF slot per batch, [128, kc, tokens-in-batch],
        # loaded token-major in [128,512] pieces so consumers start early
        x_sb = {}

        def load_x(b, q, tcns=range(4)):
            if b not in x_sb:
                x_sb[b] = xpool.tile([128, 8, S], F16, tag="xb", name="x_sb")
            for i, tcn in enumerate(tcns):
                for kc in range(8):
                    q[(i * 8 + kc) % len(q)].dma_start(
                        out=x_sb[b][:, kc, tcn * 512:(tcn + 1) * 512],
                        in_=xT_d[kc * 128:(kc + 1) * 128,
                                 b * S + tcn * 512:b * S + (tcn + 1) * 512])

        # ---- building blocks ----
        def qk_part(kind, b, tcn, part, state):
            # half a q/k projection chunk (kc 0-3 or 4-7); state carries ps
            w_sb = wq_sb if kind == "q" else wk_sb
            t_sb = qT_sb if kind == "q" else kT_sb
            if part == 0:
                state["ps"] = shp.tile([128, 512], F32, tag="sh", name="ps")
            ps = state["ps"]
            for kc in range(part * 4, part * 4 + 4):
                nc.tensor.matmul(ps[:], w_sb[:, kc, :],
                                 x_sb[b][:, kc, tcn * 512:(tcn + 1) * 512],
                                 start=kc == 0, stop=kc == 7)
            if part == 1:
                tsl = slice(b * S + tcn * 512, b * S + (tcn + 1) * 512)
                nc.vector.tensor_copy(t_sb[:, tsl], ps[:])

        def qk_chunk(kind, b, tcn):
            st = {}
            qk_part(kind, b, tcn, 0, st)
            qk_part(kind, b, tcn, 1, st)

        def v_part(b, tcn, part, state):
            # half a v chunk (kc 0-3 or 4-7); ONE accumulation group per bank
            # (start=True clears has_written bank-wide for written partitions)
            if part == 0:
                state["ps"] = shp.tile([128, 4, 128], F32, tag="sh",
                                       name="vps")
            ps = state["ps"]
            for kc in range(part * 4, part * 4 + 4):
                for sub in range(4):
                    nc.tensor.matmul(
                        ps[:, sub, :],
                        x_sb[b][:, kc, tcn * 512 + sub * 128:
                                tcn * 512 + (sub + 1) * 128],
                        wv_sb[:, kc, :],
                        start=(kc == 0 and sub == 0),
                        stop=(kc == 7 and sub == 3),
                        skip_group_check=True)
            if part == 0:
                return
            base = b * 16 + tcn * 4
            d0 = v_all[:, base, 0:HD]
            dst = bass.AP(tensor=d0.tensor, offset=d0.offset,
                          ap=[list(d0.ap[0]), [HD, 4], [32 * HD, 2], [1, HD]])
            s0 = ps[:, 0, 0:HD]
            src = bass.AP(tensor=s0.tensor, offset=s0.offset,
                          ap=[list(s0.ap[0]), [128, 4], [HD, 2], [1, HD]])
            nc.vector.tensor_copy(dst, src)

        def v_chunk(b, tcn):
            st = {}
            v_part(b, tcn, 0, st)
            v_part(b, tcn, 1, st)

        def rope_half(t_sb, b, half, rot_eng):
            # t[:, h] = t[:, h]*cos[h] + rot(t)[:, h]*sin[h] for one
            # 1024-col half of batch b.  rot = partition swap via DMA.
            hsl = slice(half * 1024, (half + 1) * 1024)
            bsl = slice(b * S + half * 1024, b * S + (half + 1) * 1024)
            rot = rotp.tile([128, 1024], F16, tag="rot", name="rot")
            nc.sync.dma_start(out=rot[0:32, :], in_=t_sb[32:64, bsl])
            nc.gpsimd.dma_start(out=rot[32:64, :], in_=t_sb[0:32, bsl])
            nc.sync.dma_start(out=rot[64:96, :], in_=t_sb[96:128, bsl])
            nc.gpsimd.dma_start(out=rot[96:128, :], in_=t_sb[64:96, bsl])
            rot_eng.tensor_mul(rot[:], rot[:], sin_sb[:, hsl])
            nc.vector.tensor_mul(t_sb[:, bsl], t_sb[:, bsl], cos_sb[:, hsl])
            nc.vector.tensor_add(t_sb[:, bsl], t_sb[:, bsl], rot[:])

        ot_cache = {}

        def proj_unit(b, qb, oc, pool=None, tag="sh", dmaq=nc.gpsimd):
            src = ctx0_sb if b == 0 else ctx1_sb
            qsl = slice(qb * 128, (qb + 1) * 128)
            osl = slice(oc * 512, (oc + 1) * 512)
            ps = (pool or shp).tile([128, 512], F32, tag=tag, name="pps")
            nc.tensor.matmul(ps[:], src[:, qsl], wo_sb[:, osl],
                             start=True, stop=True)
            if oc == 0:
                ot_cache[(b, qb)] = opool.tile([128, HID], F16, tag="ot",
                                               name="ot")
            ot = ot_cache[(b, qb)]
            nc.vector.tensor_copy(ot[:, osl], ps[:])
            if oc == 1:
                r0 = b * S + qb * 128
                dmaq.dma_start(out=out_d[r0:r0 + 128, :], in_=ot[:])
                del ot_cache[(b, qb)]

        # ---- deadline-tagged work queue ----
        # item: (cost_ns, deadline (qi, kb), emit_fn); FIFO order preserved.
        # Each drain() call adds `budget` of PE credit; items emit when
        # enough credit accumulated OR their deadline is due.
        work = []
        credit = [0]

        def drain(now, budget):
            credit[0] = min(credit[0] + budget, 6000)
            while work:
                cost, dl, fn = work[0]
                if dl > now and credit[0] < cost:
                    break
                work.pop(0)
                fn()
                credit[0] = max(credit[0] - cost, -6000)
        END = (9, 0)

        # ---- lead-in: first half of b0's keys + qc0 queries + v chunk 0;
        # Q1's kb 0-7 only need kT cols 0-1023, so attention starts early ----
        with nc.named_scope("lead"):
            load_x(0, [nc.sync, nc.gpsimd, nc.scalar], tcns=(0, 1))
            qk_chunk("k", 0, 0)
            qk_chunk("k", 0, 1)
            load_x(0, [nc.sync, nc.gpsimd, nc.scalar], tcns=(2, 3))
            rope_half(kT_sb, 0, 0, nc.gpsimd)
            qk_chunk("q", 0, 0)
            qk_chunk("q", 0, 1)
            rope_half(qT_sb, 0, 0, nc.vector)
            v_chunk(0, 0)

        # ---- queue everything else as split pieces with deadlines ----
        A = work.append
        CH, HF, PR, RP = 2600, 1300, 330, 1100

        def q_chunk_items(kind, b, tcn, dl0, dl1):
            st = {}
            A((HF, dl0, lambda: qk_part(kind, b, tcn, 0, st)))
            A((HF, dl1, lambda: qk_part(kind, b, tcn, 1, st)))

        def q_v_items(b, tcn, dl0, dl1):
            st = {}
            A((HF, dl0, lambda: v_part(b, tcn, 0, st)))
            A((HF, dl1, lambda: v_part(b, tcn, 1, st)))

        q_v_items(0, 1, (0, 1), (0, 2))
        q_chunk_items("k", 0, 2, (0, 3), (0, 4))
        q_chunk_items("k", 0, 3, (0, 5), (0, 6))
        A((RP, (0, 7), lambda: rope_half(kT_sb, 0, 1, nc.gpsimd)))
        q_v_items(0, 2, (0, 8), (0, 9))
        q_chunk_items("q", 0, 2, (0, 11), (0, 12))
        q_v_items(0, 3, (0, 13), (0, 14))
        q_chunk_items("q", 0, 3, (0, 15), (0, 16))
        A((RP, (1, 0), lambda: rope_half(qT_sb, 0, 1, nc.vector)))
        A((200, (1, 0), lambda: load_x(1, [nc.sync, nc.gpsimd])))
        for t in range(4):
            q_chunk_items("k", 1, t, (1, 3 + 3 * t), (1, 4 + 3 * t))
        A((RP, (2, 0), lambda: rope_half(kT_sb, 1, 0, nc.gpsimd)))
        A((RP, (2, 0), lambda: rope_half(kT_sb, 1, 1, nc.gpsimd)))
        q_chunk_items("q", 1, 0, (2, 0), (2, 0))
        q_chunk_items("q", 1, 1, (2, 0), (2, 0))
        A((RP, (2, 0), lambda: rope_half(qT_sb, 1, 0, nc.vector)))
        q_v_items(1, 0, (2, 0), (2, 0))
        q_v_items(1, 1, (2, 2), (2, 3))
        q_v_items(1, 2, (2, 6), (2, 7))
        q_v_items(1, 3, (2, 10), (2, 11))
        q_chunk_items("q", 1, 2, (3, 0), (3, 0))
        q_chunk_items("q", 1, 3, (3, 0), (3, 0))
        A((RP, (3, 0), lambda: rope_half(qT_sb, 1, 1, nc.vector)))

        def queue_proj(b, qb_range):
            for qb in qb_range:
                for oc in range(2):
                    A((PR, END, lambda a=qb, o=oc, bb=b: proj_unit(bb, a, o)))

        # ---- attention quarters ----
        for qi, (b, qc) in enumerate([(0, 0), (0, 1), (1, 0), (1, 1)]):
            ctx_sb = ctx0_sb if b == 0 else ctx1_sb
            # previous quarters' proj becomes available now (Q1's deferred
            # to Q3 so Q2's slack stays for b1's q projections)
            if qi == 2:
                queue_proj(0, range(8))
                queue_proj(0, range(8, 16))
            elif qi == 3:
                queue_proj(1, range(8))
            with nc.named_scope(f"attn{qi}"):
                q0 = b * S + qc * 1024
                csl = slice(qc * 1024, (qc + 1) * 1024)
                ctx_t = ctxp.tile([128, 1024], F32, tag="ctx", name="ctx_t")
                l_t = lpp.tile([128, 512], F32, tag="l", name="l_t")
                pring = {}
                for kb in range(18):
                    # deadlines must fire BEFORE this kb's attention work is
                    # emitted (emission order defines dependency direction)
                    drain((qi, kb), 700)
                    if kb < 16:
                        k0 = b * S + kb * 128
                        ksl = slice(k0, k0 + 128)
                        sp0 = spp.tile([128, 1024], F32, tag="sp0", name="sp0")
                        sp1 = spp.tile([128, 1024], F32, tag="sp1", name="sp1")
                        for half in range(2):
                            qsl = slice(q0 + half * 512, q0 + (half + 1) * 512)
                            hsl = slice(half * 512, (half + 1) * 512)
                            nc.tensor.matmul(sp0[:, hsl], kT_sb[0:64, ksl],
                                             qT_sb[0:64, qsl],
                                             start=True, stop=True)
                            nc.tensor.matmul(sp1[:, hsl], kT_sb[64:128, ksl],
                                             qT_sb[64:128, qsl],
                                             start=True, stop=True)
                        p0 = ppool.tile([128, 1024], F16, tag="p0", name="p0")
                        p1 = ppool.tile([128, 1024], F16, tag="p1", name="p1")
                        nc.scalar.activation(
                            out=p0[:], in_=sp0[:],
                            func=mybir.ActivationFunctionType.Exp)
                        nc.scalar.activation(
                            out=p1[:], in_=sp1[:],
                            func=mybir.ActivationFunctionType.Exp)
                        pring[kb] = (p0, p1)
                        if dbg is not None and qi == 0 and kb == 0:
                            dp = opool.tile([128, 1024], F32, tag="dbgp", bufs=1)
                            nc.vector.tensor_copy(dp[:], p0[:])
                            nc.sync.dma_start(out=dbg["p0"][:], in_=dp[:])
                    if kb >= 2:
                        kv = kb - 2
                        p0, p1 = pring.pop(kv)
                        sl0 = b * 16 + kv
                        sl1 = 32 + b * 16 + kv
                        st, sp_ = kv == 0, kv == 15
                        for half in range(2):
                            hsl = slice(half * 512, (half + 1) * 512)
                            nc.tensor.matmul(ctx_t[0:64, hsl],
                                             v_all[:, sl0, :], p0[:, hsl],
                                             start=st, stop=sp_)
                            nc.tensor.matmul(ctx_t[64:128, hsl],
                                             v_all[:, sl1, :], p1[:, hsl],
                                             start=st, stop=sp_)
                        nc.tensor.matmul(l_t[0:32, :], ones_sb[:],
                                         p0[:, 0:512], start=st, stop=sp_,
                                         tile_position=(0, 0))
                        nc.tensor.matmul(l_t[32:64, :], ones_sb[:],
                                         p0[:, 512:1024], start=st, stop=sp_,
                                         tile_position=(0, 32))
                        nc.tensor.matmul(l_t[64:96, :], ones_sb[:],
                                         p1[:, 0:512], start=st, stop=sp_,
                                         tile_position=(0, 64))
                        nc.tensor.matmul(l_t[96:128, :], ones_sb[:],
                                         p1[:, 512:1024], start=st, stop=sp_,
                                         tile_position=(0, 96))

                # ---- normalize this quarter ----
                cuh = lpool.tile([128, 1024], F32, tag="cu", name="cuh")
                nc.vector.tensor_copy(cuh[:], ctx_t[:])
                if dbg is not None and qi == 0:
                    nc.sync.dma_start(out=dbg["ctxq1"][:], in_=cuh[:])
                lsb = lpool.tile([128, 512], F32, tag="ls", name="lsb")
                nc.vector.tensor_copy(lsb[:], l_t[:])
                lrec = lpool.tile([128, 512], F32, tag="lr", name="lrec")
                nc.vector.reciprocal_approx_fast(out=lrec[:], in_=lsb[:])
                if dbg is not None and qi == 0:
                    nc.sync.dma_start(out=dbg["lq"][:], in_=lrec[:])
                if dbg is not None and qi == 1:
                    nc.sync.dma_start(out=dbg["ls2"][:], in_=lsb[:])
                    nc.sync.dma_start(out=dbg["lq2"][:], in_=lrec[:])
                nq = nc.scalar if qi == 3 else nc.sync
                nq.dma_start(out=rscr_d[qi, 0:97, :], in_=lrec[0:97, :])
                bct = bpool.tile([128, 1024], F32, tag="bct", name="bct")
                for h in range(2):
                    for half in range(2):
                        rr = rscr_d[qi, (h * 2 + half) * 32, :]
                        nq.dma_start(
                            out=bct[h * 64:(h + 1) * 64,
                                    half * 512:(half + 1) * 512],
                            in_=bass.AP(tensor=rr.tensor, offset=rr.offset,
                                        ap=[[0, 64], [1, 512]]))
                if dbg is not None and qi == 0:
                    nc.sync.dma_start(out=dbg["bct1"][:], in_=bct[:])
                nc.vector.tensor_mul(ctx_sb[0:64, csl], cuh[0:64, :],
                                     bct[0:64, :])
                nc.vector.tensor_mul(ctx_sb[64:128, csl], cuh[64:128, :],
                                     bct[64:128, :])

        # ---- tail: drain leftovers, then Q4's proj over 4 psum slots ----
        with nc.named_scope("tail"):
            drain(END, 10**9)
            units = [(1, qb, oc) for qb in range(8, 16) for oc in range(2)]
            pools = [(spp, "sp0"), (spp, "sp1"), (ctxp, "ctx"), (shp, "sh")]
            for i, (b_, qb, oc) in enumerate(units):
                pl, tg = pools[i % 4]
                proj_unit(b_, qb, oc, pool=pl, tag=tg,
                          dmaq=(nc.sync, nc.gpsimd, nc.scalar)[i % 3])

        if dbg is not None:
            nc.gpsimd.dma_start(out=dbg["qT"][:], in_=qT_sb[:])
            nc.gpsimd.dma_start(out=dbg["kT"][:], in_=kT_sb[:])
            dv = opool.tile([128, 64], F32, tag="dbgv", bufs=1)
            nc.vector.tensor_copy(dv[:], v_all[:, 0, :])
            nc.sync.dma_start(out=dbg["v0"][:], in_=dv[:])
            dva = opool.tile([128, 64 * HD], F32, tag="dbgva", bufs=1)
            nc.vector.tensor_copy(dva[:], v_all[:].rearrange("p a b -> p (a b)"))
            nc.sync.dma_start(out=dbg["vall"][:], in_=dva[:])
            dc = opool.tile([128, S], F32, tag="dbgc", bufs=1)
            nc.vector.tensor_copy(dc[:], ctx0_sb[:])
            nc.sync.dma_start(out=dbg["ctx0"][:], in_=dc[:])


def _swz(w):
    # [1024, 128] -> [128, 1024]: SBUF layout [p, kc*128+d] = w[kc*128+p, d]
    return np.ascontiguousarray(
        w.reshape(8, 128, 128).transpose(1, 0, 2).reshape(128, 1024))


def _prep_inputs(x, Wq, Wk, Wv, Wo):
    x2 = np.asarray(x, dtype=np.float32).reshape(T, HID)
    xT16 = np.ascontiguousarray(x2.T).astype(np.float16)

    half = HD // 2
    inv_freq = (1.0 / (ROPE_BASE ** (np.arange(half, dtype=np.float64) * 2.0 / HD)))
    ang = np.arange(S, dtype=np.float64)[None, :] * inv_freq[:, None]  # [32, S]
    cosf = np.tile(np.cos(ang), (4, 1)).astype(np.float16)
    sgn = np.repeat([-1.0, 1.0, -1.0, 1.0], 32)[:, None]
    sins = (np.tile(np.sin(ang), (4, 1)) * sgn).astype(np.float16)

    scale = np.float32(1.0 / np.sqrt(HD))
    in_maps = []
    for c in range(NCORES):
        rows = slice(c * DPC, (c + 1) * DPC)
        in_maps.append({
            "xT16": xT16,
            "wq": _swz((Wq[rows, :] * scale).T.astype(np.float16)),
            "wk": _swz(Wk[rows, :].T.astype(np.float16)),
            "wv": _swz(Wv[rows, :].T.astype(np.float16)),
            "wo": np.ascontiguousarray(Wo[:, rows].T).astype(np.float16),
            "cosf": cosf,
            "sins": sins,
        })
    return in_maps


def _run(in_maps, trace=False):
    if "nc" not in _CACHE:
        _CACHE["nc"] = _build_program()
    nc = _CACHE["nc"]
    res = run_bass_kernel_spmd(nc, in_maps, core_ids=list(range(NCORES)),
                               trace=trace)
    acc = res.results[0]["out"].astype(np.float32).copy()
    for c in range(1, NCORES):
        acc += res.results[c]["out"].astype(np.float32)
    return acc.reshape(B, S, HID), res


def kernel(x, Wq, Wk, Wv, Wo):
    in_maps = _prep_inputs(np.asarray(x), np.asarray(Wq), np.asarray(Wk),
                           np.asarray(Wv), np.asarray(Wo))
    out, _ = _run(in_maps, trace=False)
    return out


def run_profiled(x, Wq, Wk, Wv, Wo):
    in_maps = _prep_inputs(np.asarray(x), np.asarray(Wq), np.asarray(Wk),
                           np.asarray(Wv), np.asarray(Wo))
    return _run(in_maps, trace=True)


# revision 21
# speedup vs baseline: 1.1640x; 1.1031x over previous
"""Multi-head attention with RoPE (B=2, S=2048, H=16, D=64) on 8 TRN2 cores,
tensor-parallel over heads (2 heads/core); host sums the 8 rank-128 fp16
partial outputs.

Restructured from the 252us baseline around two engine budgets:
  - ScalarE runs ONLY the 128 softmax exps ([128,1024] each, ~1.1us);
    everything else (copies, DMA issue) lives on DVE/Sync/GpSimd queues.
  - PE work cut via tile_position packing:
      * scores: h0 (rows 0-63) and h1 (rows 64-127) issue as adjacent
        K=64 matmuls on disjoint row-groups -> run concurrently.
      * PV: v has no ones column (M=64); h0 writes ctx psum partitions
        0-63, h1 partitions 64-127 (col tile (0,64)), concurrently from
        the two p streams.  start=True clears has_written for the whole
        bank but only on the written partitions, so partition-disjoint
        groups are safe (free-offset-disjoint groups are NOT - see
        v_chunk, which uses a single accumulation group per bank).
      * softmax denominators via an "l-quad": four M=1 ones-stationary
        matmuls col-tiled at psum partitions 0/32/64/96, accumulating
        sum_k exp over the kb loop.
  - Single fused timeline: lead-in computes k(b0), rope-k, q(b0) first
    half, rope-q half0, v(b0) chunk0; everything else (rest of b0,
    qkv(b1), output projection) drains from a deadline-tagged work queue
    inside the attention kb loops, paced to fill PE slack under the
    ScalarE exp cadence.
  - PSUM (8 banks): sp0 2 + sp1 2 + packed ctx 2 + l 1 + one shared slot
    (qkv-psum / v-psum / proj) 1.
  - x cached in SBUF per batch (1 HBM read), loaded in [128,512] pieces
    token-major so the first k-projection starts after ~1 piece.
  - outputs stored fp16 (host sums in fp32).
"""
import numpy as np
import ml_dtypes

import concourse.bass as bass
import concourse.mybir as mybir
import concourse.tile as tile
from concourse import bacc
from concourse.bass_utils import run_bass_kernel_spmd

F32 = mybir.dt.float32
F16 = mybir.dt.float16

B, S, HID = 2, 2048, 1024
NH, HD = 16, 64
T = B * S
NCORES = 8
HPC = NH // NCORES         # 2 heads per core
DPC = HPC * HD             # 128 context dims per core
ROPE_BASE = 10000.0

_CACHE = {}
DEBUG_TAPS = False


def _build_program():
    nc = bacc.Bacc("TRN2", target_bir_lowering=False, debug=False)

    xT_d = nc.dram_tensor("xT16", [HID, T], F16, kind="ExternalInput")
    wq_d = nc.dram_tensor("wq", [128, HID], F16, kind="ExternalInput")
    wk_d = nc.dram_tensor("wk", [128, HID], F16, kind="ExternalInput")
    wv_d = nc.dram_tensor("wv", [128, HID], F16, kind="ExternalInput")
    wo_d = nc.dram_tensor("wo", [DPC, HID], F16, kind="ExternalInput")
    cos_d = nc.dram_tensor("cosf", [128, S], F16, kind="ExternalInput")
    sin_d = nc.dram_tensor("sins", [128, S], F16, kind="ExternalInput")
    out_d = nc.dram_tensor("out", [T, HID], F16, kind="ExternalOutput")
    rscr_d = nc.dram_tensor("rscr", [4, 128, 512], F32,
                            kind="ExternalOutput" if DEBUG_TAPS else "Internal")

    dbg = None
    if DEBUG_TAPS:
        dbg = {
            "qT": nc.dram_tensor("dbg_qT", [128, T], F32, kind="ExternalOutput"),
            "kT": nc.dram_tensor("dbg_kT", [128, T], F32, kind="ExternalOutput"),
            "v0": nc.dram_tensor("dbg_v0", [128, 64], F32, kind="ExternalOutput"),
            "p0": nc.dram_tensor("dbg_p0", [128, 1024], F32, kind="ExternalOutput"),
            "lq": nc.dram_tensor("dbg_lq", [128, 512], F32, kind="ExternalOutput"),
            "ls2": nc.dram_tensor("dbg_ls2", [128, 512], F32, kind="ExternalOutput"),
            "lq2": nc.dram_tensor("dbg_lq2", [128, 512], F32, kind="ExternalOutput"),
            "ctx0": nc.dram_tensor("dbg_ctx0", [128, S], F32, kind="ExternalOutput"),
            "vall": nc.dram_tensor("dbg_vall", [128, 64 * HD], F32, kind="ExternalOutput"),
            "ctxq1": nc.dram_tensor("dbg_ctxq1", [128, 1024], F32, kind="ExternalOutput"),
            "bct1": nc.dram_tensor("dbg_bct1", [128, 1024], F32, kind="ExternalOutput"),
        }

    with tile.TileContext(nc) as tc:
        _emit(nc, tc, xT_d, wq_d, wk_d, wv_d, wo_d, cos_d, sin_d, out_d,
              rscr_d, dbg=dbg)
    nc.compile()
    return nc


def _emit(nc, tc, xT_d, wq_d, wk_d, wv_d, wo_d, cos_d, sin_d, out_d,
          rscr_d, dbg=None):
    import contextlib
    ctx = contextlib.ExitStack()
    with ctx:
        singles = ctx.enter_context(tc.tile_pool(name="singles", bufs=1))
        xpool = ctx.enter_context(tc.tile_pool(name="xpool", bufs=2))
        ppool = ctx.enter_context(tc.tile_pool(name="ppool", bufs=3))
        rotp = ctx.enter_context(tc.tile_pool(name="rotp", bufs=3))
        lpool = ctx.enter_context(tc.tile_pool(name="lpool", bufs=2))
        bpool = ctx.enter_context(tc.tile_pool(name="bpool", bufs=2))
        opool = ctx.enter_context(tc.tile_pool(name="opool", bufs=3))
        # PSUM: exactly 8 banks
        spp = ctx.enter_context(tc.tile_pool(name="spp", bufs=1, space="PSUM"))
        ctxp = ctx.enter_context(tc.tile_pool(name="ctxp", bufs=1, space="PSUM"))
        lpp = ctx.enter_context(tc.tile_pool(name="lpp", bufs=1, space="PSUM"))
        shp = ctx.enter_context(tc.tile_pool(name="shp", bufs=1, space="PSUM"))

        # ---- persistent SBUF ----
        wq_sb = singles.tile([128, 8, DPC], F16)
        wk_sb = singles.tile([128, 8, DPC], F16)
        wv_sb = singles.tile([128, 8, DPC], F16)
        wo_sb = singles.tile([128, HID], F16)
        cos_sb = singles.tile([128, S], F16)
        sin_sb = singles.tile([128, S], F16)
        qT_sb = singles.tile([128, T], F16)
        kT_sb = singles.tile([128, T], F16)
        v_all = singles.tile([128, 64, HD], F16)   # slot = h*32 + b*16 + kb
        ctx0_sb = singles.tile([128, S], F16)      # normalized ctx^T, b=0
        ctx1_sb = singles.tile([128, S], F16)
        ones_sb = singles.tile([128, 32], F16)

        nc.sync.dma_start(out=wq_sb[:].rearrange("p a b -> p (a b)"), in_=wq_d[:])
        nc.scalar.dma_start(out=wk_sb[:].rearrange("p a b -> p (a b)"), in_=wk_d[:])
        nc.gpsimd.dma_start(out=wv_sb[:].rearrange("p a b -> p (a b)"), in_=wv_d[:])
        nc.gpsimd.dma_start(out=wo_sb[:], in_=wo_d[:])
        nc.scalar.dma_start(out=cos_sb[:], in_=cos_d[:])
        nc.scalar.dma_start(out=sin_sb[:], in_=sin_d[:])
        nc.vector.memset(ones_sb[:], 1.0)
        # preload the exp table set during the lead-in
        tblw = singles.tile([1, 8], F32)
        nc.vector.memset(tblw[:], 0.0)
        nc.scalar.activation(out=tblw[:], in_=tblw[:],
                             func=mybir.ActivationFunctionType.Exp)

        # x cache: one SBUF slot per batch, [128, kc, tokens-in-batch],
        # loaded token-major in [128,512] pieces so consumers start early
        x_sb = {}

        def load_x(b, q, tcns=range(4)):
            if b not in x_sb:
                x_sb[b] = xpool.tile([128, 8, S], F16, tag="xb", name="x_sb")
            for i, tcn in enumerate(tcns):
                for kc in range(8):
                    q[(i * 8 + kc) % len(q)].dma_start(
                        out=x_sb[b][:, kc, tcn * 512:(tcn + 1) * 512],
                        in_=xT_d[kc * 128:(kc + 1) * 128,
                                 b * S + tcn * 512:b * S + (tcn + 1) * 512])

        # ---- building blocks ----
        def qk_part(kind, b, tcn, part, state):
            # half a q/k projection chunk (kc 0-3 or 4-7); state carries ps
            w_sb = wq_sb if kind == "q" else wk_sb
            t_sb = qT_sb if kind == "q" else kT_sb
            if part == 0:
                state["ps"] = shp.tile([128, 512], F32, tag="sh", name="ps")
            ps = state["ps"]
            for kc in range(part * 4, part * 4 + 4):
                nc.tensor.matmul(ps[:], w_sb[:, kc, :],
                                 x_sb[b][:, kc, tcn * 512:(tcn + 1) * 512],
                                 start=kc == 0, stop=kc == 7)
            if part == 1:
                tsl = slice(b * S + tcn * 512, b * S + (tcn + 1) * 512)
                nc.vector.tensor_copy(t_sb[:, tsl], ps[:])

        def qk_chunk(kind, b, tcn):
            st = {}
            qk_part(kind, b, tcn, 0, st)
            qk_part(kind, b, tcn, 1, st)

        def v_part(b, tcn, part, state):
            # half a v chunk (kc 0-3 or 4-7); ONE accumulation group per bank
            # (start=True clears has_written bank-wide for written partitions)
            if part == 0:
                state["ps"] = shp.tile([128, 4, 128], F32, tag="sh",
                                       name="vps")
            ps = state["ps"]
            for kc in range(part * 4, part * 4 + 4):
                for sub in range(4):
                    nc.tensor.matmul(
                        ps[:, sub, :],
                        x_sb[b][:, kc, tcn * 512 + sub * 128:
                                tcn * 512 + (sub + 1) * 128],
                        wv_sb[:, kc, :],
                        start=(kc == 0 and sub == 0),
                        stop=(kc == 7 and sub == 3),
                        skip_group_check=True)
            if part == 0:
                return
            base = b * 16 + tcn * 4
            d0 = v_all[:, base, 0:HD]
            dst = bass.AP(tensor=d0.tensor, offset=d0.offset,
                          ap=[list(d0.ap[0]), [HD, 4], [32 * HD, 2], [1, HD]])
            s0 = ps[:, 0, 0:HD]
            src = bass.AP(tensor=s0.tensor, offset=s0.offset,
                          ap=[list(s0.ap[0]), [128, 4], [HD, 2], [1, HD]])
            nc.vector.tensor_copy(dst, src)

        def v_chunk(b, tcn):
            st = {}
            v_part(b, tcn, 0, st)
            v_part(b, tcn, 1, st)

        def rope_half(t_sb, b, half, rot_eng):
            # t[:, h] = t[:, h]*cos[h] + rot(t)[:, h]*sin[h] for one
            # 1024-col half of batch b.  rot = partition swap via DMA.
            hsl = slice(half * 1024, (half + 1) * 1024)
            bsl = slice(b * S + half * 1024, b * S + (half + 1) * 1024)
            rot = rotp.tile([128, 1024], F16, tag="rot", name="rot")
            nc.sync.dma_start(out=rot[0:32, :], in_=t_sb[32:64, bsl])
            nc.gpsimd.dma_start(out=rot[32:64, :], in_=t_sb[0:32, bsl])
            nc.sync.dma_start(out=rot[64:96, :], in_=t_sb[96:128, bsl])
            nc.gpsimd.dma_start(out=rot[96:128, :], in_=t_sb[64:96, bsl])
            rot_eng.tensor_mul(rot[:], rot[:], sin_sb[:, hsl])
            nc.vector.tensor_mul(t_sb[:, bsl], t_sb[:, bsl], cos_sb[:, hsl])
            nc.vector.tensor_add(t_sb[:, bsl], t_sb[:, bsl], rot[:])

        ot_cache = {}

        def proj_unit(b, qb, oc, pool=None, tag="sh", dmaq=nc.gpsimd):
            src = ctx0_sb if b == 0 else ctx1_sb
            qsl = slice(qb * 128, (qb + 1) * 128)
            osl = slice(oc * 512, (oc + 1) * 512)
            ps = (pool or shp).tile([128, 512], F32, tag=tag, name="pps")
            nc.tensor.matmul(ps[:], src[:, qsl], wo_sb[:, osl],
                             start=True, stop=True)
            if oc == 0:
                ot_cache[(b, qb)] = opool.tile([128, HID], F16, tag="ot",
                                               name="ot")
            ot = ot_cache[(b, qb)]
            nc.vector.tensor_copy(ot[:, osl], ps[:])
            if oc == 1:
                r0 = b * S + qb * 128
                dmaq.dma_start(out=out_d[r0:r0 + 128, :], in_=ot[:])
                del ot_cache[(b, qb)]

        # ---- deadline-tagged work queue ----
        # item: (cost_ns, deadline (qi, kb), emit_fn); FIFO order preserved.
        # Each drain() call adds `budget` of PE credit; items emit when
        # enough credit accumulated OR their deadline is due.
        work = []
        credit = [0]

        def drain(now, budget):
            credit[0] = min(credit[0] + budget, 6000)
            while work:
                cost, dl, fn = work[0]
                if dl > now and credit[0] < cost:
                    break
                work.pop(0)
                fn()
                credit[0] = max(credit[0] - cost, -6000)
        END = (9, 0)

        # ---- lead-in: full qkv for both batches, DMA-pipelined ----
        with nc.named_scope("lead"):
            load_x(0, [nc.sync, nc.gpsimd, nc.scalar])
            load_x(1, [nc.sync, nc.gpsimd, nc.scalar])
            qk_chunk("k", 0, 0)
            qk_chunk("k", 0, 1)
            qk_chunk("q", 0, 0)
            qk_chunk("q", 0, 1)
            rope_half(kT_sb, 0, 0, nc.gpsimd)
            rope_half(qT_sb, 0, 0, nc.vector)
            v_chunk(0, 0)
            v_chunk(0, 1)
            qk_chunk("k", 0, 2)
            qk_chunk("k", 0, 3)
            qk_chunk("q", 0, 2)
            qk_chunk("q", 0, 3)
            rope_half(kT_sb, 0, 1, nc.gpsimd)
            rope_half(qT_sb, 0, 1, nc.vector)
            v_chunk(0, 2)
            v_chunk(0, 3)
            qk_chunk("k", 1, 0)
            qk_chunk("k", 1, 1)
            qk_chunk("q", 1, 0)
            qk_chunk("q", 1, 1)
            rope_half(kT_sb, 1, 0, nc.gpsimd)
            rope_half(qT_sb, 1, 0, nc.vector)
            v_chunk(1, 0)
            v_chunk(1, 1)
            qk_chunk("k", 1, 2)
            qk_chunk("k", 1, 3)
            qk_chunk("q", 1, 2)
            qk_chunk("q", 1, 3)
            rope_half(kT_sb, 1, 1, nc.gpsimd)
            rope_half(qT_sb, 1, 1, nc.vector)
            v_chunk(1, 2)
            v_chunk(1, 3)

        A = work.append
        CH, HF, PR, RP = 2600, 1300, 330, 1100

        def queue_proj(b, qb_range):
            for qb in qb_range:
                for oc in range(2):
                    A((PR, END, lambda a=qb, o=oc, bb=b: proj_unit(bb, a, o)))

        # ---- attention quarters ----
        for qi, (b, qc) in enumerate([(0, 0), (0, 1), (1, 0), (1, 1)]):
            ctx_sb = ctx0_sb if b == 0 else ctx1_sb
            # previous quarters' proj becomes available now (Q1's deferred
            # to Q3 so Q2's slack stays for b1's q projections)
            if qi == 2:
                queue_proj(0, range(8))
                queue_proj(0, range(8, 16))
            elif qi == 3:
                queue_proj(1, range(8))
            with nc.named_scope(f"attn{qi}"):
                q0 = b * S + qc * 1024
                csl = slice(qc * 1024, (qc + 1) * 1024)
                ctx_t = ctxp.tile([128, 1024], F32, tag="ctx", name="ctx_t")
                l_t = lpp.tile([128, 512], F32, tag="l", name="l_t")
                pring = {}
                for kb in range(18):
                    # deadlines must fire BEFORE this kb's attention work is
                    # emitted (emission order defines dependency direction)
                    drain((qi, kb), 700)
                    if kb < 16:
                        k0 = b * S + kb * 128
                        ksl = slice(k0, k0 + 128)
                        sp0 = spp.tile([128, 1024], F32, tag="sp0", name="sp0")
                        sp1 = spp.tile([128, 1024], F32, tag="sp1", name="sp1")
                        for half in range(2):
                            qsl = slice(q0 + half * 512, q0 + (half + 1) * 512)
                            hsl = slice(half * 512, (half + 1) * 512)
                            nc.tensor.matmul(sp0[:, hsl], kT_sb[0:64, ksl],
                                             qT_sb[0:64, qsl],
                                             start=True, stop=True)
                            nc.tensor.matmul(sp1[:, hsl], kT_sb[64:128, ksl],
                                             qT_sb[64:128, qsl],
                                             start=True, stop=True)
                        p0 = ppool.tile([128, 1024], F16, tag="p0", name="p0")
                        p1 = ppool.tile([128, 1024], F16, tag="p1", name="p1")
                        nc.scalar.activation(
                            out=p0[:], in_=sp0[:],
                            func=mybir.ActivationFunctionType.Exp)
                        nc.scalar.activation(
                            out=p1[:], in_=sp1[:],
                            func=mybir.ActivationFunctionType.Exp)
                        pring[kb] = (p0, p1)
                        if dbg is not None and qi == 0 and kb == 0:
                            dp = opool.tile([128, 1024], F32, tag="dbgp", bufs=1)
                            nc.vector.tensor_copy(dp[:], p0[:])
                            nc.sync.dma_start(out=dbg["p0"][:], in_=dp[:])
                    if kb >= 2:
                        kv = kb - 2
                        p0, p1 = pring.pop(kv)
                        sl0 = b * 16 + kv
                        sl1 = 32 + b * 16 + kv
                        st, sp_ = kv == 0, kv == 15
                        for half in range(2):
                            hsl = slice(half * 512, (half + 1) * 512)
                            nc.tensor.matmul(ctx_t[0:64, hsl],
                                             v_all[:, sl0, :], p0[:, hsl],
                                             start=st, stop=sp_)
                            nc.tensor.matmul(ctx_t[64:128, hsl],
                                             v_all[:, sl1, :], p1[:, hsl],
                                             start=st, stop=sp_)
                        nc.tensor.matmul(l_t[0:32, :], ones_sb[:],
                                         p0[:, 0:512], start=st, stop=sp_,
                                         tile_position=(0, 0))
                        nc.tensor.matmul(l_t[32:64, :], ones_sb[:],
                                         p0[:, 512:1024], start=st, stop=sp_,
                                         tile_position=(0, 32))
                        nc.tensor.matmul(l_t[64:96, :], ones_sb[:],
                                         p1[:, 0:512], start=st, stop=sp_,
                                         tile_position=(0, 64))
                        nc.tensor.matmul(l_t[96:128, :], ones_sb[:],
                                         p1[:, 512:1024], start=st, stop=sp_,
                                         tile_position=(0, 96))

                # ---- normalize this quarter ----
                cuh = lpool.tile([128, 1024], F32, tag="cu", name="cuh")
                nc.vector.tensor_copy(cuh[:], ctx_t[:])
                if dbg is not None and qi == 0:
                    nc.sync.dma_start(out=dbg["ctxq1"][:], in_=cuh[:])
                lsb = lpool.tile([128, 512], F32, tag="ls", name="lsb")
                nc.vector.tensor_copy(lsb[:], l_t[:])
                lrec = lpool.tile([128, 512], F32, tag="lr", name="lrec")
                nc.vector.reciprocal_approx_fast(out=lrec[:], in_=lsb[:])
                if dbg is not None and qi == 0:
                    nc.sync.dma_start(out=dbg["lq"][:], in_=lrec[:])
                if dbg is not None and qi == 1:
                    nc.sync.dma_start(out=dbg["ls2"][:], in_=lsb[:])
                    nc.sync.dma_start(out=dbg["lq2"][:], in_=lrec[:])
                nq = nc.scalar if qi == 3 else nc.sync
                nq.dma_start(out=rscr_d[qi, 0:97, :], in_=lrec[0:97, :])
                bct = bpool.tile([128, 1024], F32, tag="bct", name="bct")
                for h in range(2):
                    for half in range(2):
                        rr = rscr_d[qi, (h * 2 + half) * 32, :]
                        nq.dma_start(
                            out=bct[h * 64:(h + 1) * 64,
                                    half * 512:(half + 1) * 512],
                            in_=bass.AP(tensor=rr.tensor, offset=rr.offset,
                                        ap=[[0, 64], [1, 512]]))
                if dbg is not None and qi == 0:
                    nc.sync.dma_start(out=dbg["bct1"][:], in_=bct[:])
                nc.vector.tensor_mul(ctx_sb[0:64, csl], cuh[0:64, :],
                                     bct[0:64, :])
                nc.vector.tensor_mul(ctx_sb[64:128, csl], cuh[64:128, :],
                                     bct[64:128, :])

        # ---- tail: drain leftovers, then Q4's proj over 4 psum slots ----
        with nc.named_scope("tail"):
            drain(END, 10**9)
            units = [(1, qb, oc) for qb in range(8, 16) for oc in range(2)]
            pools = [(spp, "sp0"), (spp, "sp1"), (ctxp, "ctx"), (shp, "sh")]
            for i, (b_, qb, oc) in enumerate(units):
                pl, tg = pools[i % 4]
                proj_unit(b_, qb, oc, pool=pl, tag=tg,
                          dmaq=(nc.sync, nc.gpsimd, nc.scalar)[i % 3])

        if dbg is not None:
            nc.gpsimd.dma_start(out=dbg["qT"][:], in_=qT_sb[:])
            nc.gpsimd.dma_start(out=dbg["kT"][:], in_=kT_sb[:])
            dv = opool.tile([128, 64], F32, tag="dbgv", bufs=1)
            nc.vector.tensor_copy(dv[:], v_all[:, 0, :])
            nc.sync.dma_start(out=dbg["v0"][:], in_=dv[:])
            dva = opool.tile([128, 64 * HD], F32, tag="dbgva", bufs=1)
            nc.vector.tensor_copy(dva[:], v_all[:].rearrange("p a b -> p (a b)"))
            nc.sync.dma_start(out=dbg["vall"][:], in_=dva[:])
            dc = opool.tile([128, S], F32, tag="dbgc", bufs=1)
            nc.vector.tensor_copy(dc[:], ctx0_sb[:])
            nc.sync.dma_start(out=dbg["ctx0"][:], in_=dc[:])


def _swz(w):
    # [1024, 128] -> [128, 1024]: SBUF layout [p, kc*128+d] = w[kc*128+p, d]
    return np.ascontiguousarray(
        w.reshape(8, 128, 128).transpose(1, 0, 2).reshape(128, 1024))


def _prep_inputs(x, Wq, Wk, Wv, Wo):
    x2 = np.asarray(x, dtype=np.float32).reshape(T, HID)
    xT16 = np.ascontiguousarray(x2.T).astype(np.float16)

    half = HD // 2
    inv_freq = (1.0 / (ROPE_BASE ** (np.arange(half, dtype=np.float64) * 2.0 / HD)))
    ang = np.arange(S, dtype=np.float64)[None, :] * inv_freq[:, None]  # [32, S]
    cosf = np.tile(np.cos(ang), (4, 1)).astype(np.float16)
    sgn = np.repeat([-1.0, 1.0, -1.0, 1.0], 32)[:, None]
    sins = (np.tile(np.sin(ang), (4, 1)) * sgn).astype(np.float16)

    scale = np.float32(1.0 / np.sqrt(HD))
    in_maps = []
    for c in range(NCORES):
        rows = slice(c * DPC, (c + 1) * DPC)
        in_maps.append({
            "xT16": xT16,
            "wq": _swz((Wq[rows, :] * scale).T.astype(np.float16)),
            "wk": _swz(Wk[rows, :].T.astype(np.float16)),
            "wv": _swz(Wv[rows, :].T.astype(np.float16)),
            "wo": np.ascontiguousarray(Wo[:, rows].T).astype(np.float16),
            "cosf": cosf,
            "sins": sins,
        })
    return in_maps


def _run(in_maps, trace=False):
    if "nc" not in _CACHE:
        _CACHE["nc"] = _build_program()
    nc = _CACHE["nc"]
    res = run_bass_kernel_spmd(nc, in_maps, core_ids=list(range(NCORES)),
                               trace=trace)
    acc = res.results[0]["out"].astype(np.float32).copy()
    for c in range(1, NCORES):
        acc += res.results[c]["out"].astype(np.float32)
    return acc.reshape(B, S, HID), res


def kernel(x, Wq, Wk, Wv, Wo):
    in_maps = _prep_inputs(np.asarray(x), np.asarray(Wq), np.asarray(Wk),
                           np.asarray(Wv), np.asarray(Wo))
    out, _ = _run(in_maps, trace=False)
    return out


def run_profiled(x, Wq, Wk, Wv, Wo):
    in_maps = _prep_inputs(np.asarray(x), np.asarray(Wq), np.asarray(Wk),
                           np.asarray(Wv), np.asarray(Wo))
    return _run(in_maps, trace=True)


# revision 22
# speedup vs baseline: 1.2173x; 1.0458x over previous
"""Multi-head attention with RoPE (B=2, S=2048, H=16, D=64) on 8 TRN2 cores,
tensor-parallel over heads (2 heads/core); host sums the 8 rank-128 fp16
partial outputs.

Restructured from the 252us baseline around two engine budgets:
  - ScalarE runs ONLY the 128 softmax exps ([128,1024] each, ~1.1us);
    everything else (copies, DMA issue) lives on DVE/Sync/GpSimd queues.
  - PE work cut via tile_position packing:
      * scores: h0 (rows 0-63) and h1 (rows 64-127) issue as adjacent
        K=64 matmuls on disjoint row-groups -> run concurrently.
      * PV: v has no ones column (M=64); h0 writes ctx psum partitions
        0-63, h1 partitions 64-127 (col tile (0,64)), concurrently from
        the two p streams.  start=True clears has_written for the whole
        bank but only on the written partitions, so partition-disjoint
        groups are safe (free-offset-disjoint groups are NOT - see
        v_chunk, which uses a single accumulation group per bank).
      * softmax denominators via an "l-quad": four M=1 ones-stationary
        matmuls col-tiled at psum partitions 0/32/64/96, accumulating
        sum_k exp over the kb loop.
  - Single fused timeline: lead-in computes k(b0), rope-k, q(b0) first
    half, rope-q half0, v(b0) chunk0; everything else (rest of b0,
    qkv(b1), output projection) drains from a deadline-tagged work queue
    inside the attention kb loops, paced to fill PE slack under the
    ScalarE exp cadence.
  - PSUM (8 banks): sp0 2 + sp1 2 + packed ctx 2 + l 1 + one shared slot
    (qkv-psum / v-psum / proj) 1.
  - x cached in SBUF per batch (1 HBM read), loaded in [128,512] pieces
    token-major so the first k-projection starts after ~1 piece.
  - outputs stored fp16 (host sums in fp32).
"""
import numpy as np
import ml_dtypes

import concourse.bass as bass
import concourse.mybir as mybir
import concourse.tile as tile
from concourse import bacc
from concourse.bass_utils import run_bass_kernel_spmd

F32 = mybir.dt.float32
F16 = mybir.dt.float16

B, S, HID = 2, 2048, 1024
NH, HD = 16, 64
T = B * S
NCORES = 8
HPC = NH // NCORES         # 2 heads per core
DPC = HPC * HD             # 128 context dims per core
ROPE_BASE = 10000.0

_CACHE = {}
DEBUG_TAPS = False


def _build_program():
    nc = bacc.Bacc("TRN2", target_bir_lowering=False, debug=False)

    xT_d = nc.dram_tensor("xT16", [HID, T], F16, kind="ExternalInput")
    wq_d = nc.dram_tensor("wq", [128, HID], F16, kind="ExternalInput")
    wk_d = nc.dram_tensor("wk", [128, HID], F16, kind="ExternalInput")
    wv_d = nc.dram_tensor("wv", [128, HID], F16, kind="ExternalInput")
    wo_d = nc.dram_tensor("wo", [DPC, HID], F16, kind="ExternalInput")
    cos_d = nc.dram_tensor("cosf", [128, S], F16, kind="ExternalInput")
    sin_d = nc.dram_tensor("sins", [128, S], F16, kind="ExternalInput")
    out_d = nc.dram_tensor("out", [T, HID], F16, kind="ExternalOutput")
    rscr_d = nc.dram_tensor("rscr", [4, 128, 512], F32,
                            kind="ExternalOutput" if DEBUG_TAPS else "Internal")

    dbg = None
    if DEBUG_TAPS:
        dbg = {
            "qT": nc.dram_tensor("dbg_qT", [128, T], F32, kind="ExternalOutput"),
            "kT": nc.dram_tensor("dbg_kT", [128, T], F32, kind="ExternalOutput"),
            "v0": nc.dram_tensor("dbg_v0", [128, 64], F32, kind="ExternalOutput"),
            "p0": nc.dram_tensor("dbg_p0", [128, 1024], F32, kind="ExternalOutput"),
            "lq": nc.dram_tensor("dbg_lq", [128, 512], F32, kind="ExternalOutput"),
            "ls2": nc.dram_tensor("dbg_ls2", [128, 512], F32, kind="ExternalOutput"),
            "lq2": nc.dram_tensor("dbg_lq2", [128, 512], F32, kind="ExternalOutput"),
            "ctx0": nc.dram_tensor("dbg_ctx0", [128, S], F32, kind="ExternalOutput"),
            "vall": nc.dram_tensor("dbg_vall", [128, 64 * HD], F32, kind="ExternalOutput"),
            "ctxq1": nc.dram_tensor("dbg_ctxq1", [128, 1024], F32, kind="ExternalOutput"),
            "bct1": nc.dram_tensor("dbg_bct1", [128, 1024], F32, kind="ExternalOutput"),
        }

    with tile.TileContext(nc) as tc:
        _emit(nc, tc, xT_d, wq_d, wk_d, wv_d, wo_d, cos_d, sin_d, out_d,
              rscr_d, dbg=dbg)
    nc.compile()
    return nc


def _emit(nc, tc, xT_d, wq_d, wk_d, wv_d, wo_d, cos_d, sin_d, out_d,
          rscr_d, dbg=None):
    import contextlib
    ctx = contextlib.ExitStack()
    with ctx:
        singles = ctx.enter_context(tc.tile_pool(name="singles", bufs=1))
        xpool = ctx.enter_context(tc.tile_pool(name="xpool", bufs=2))
        ppool = ctx.enter_context(tc.tile_pool(name="ppool", bufs=3))
        rotp = ctx.enter_context(tc.tile_pool(name="rotp", bufs=3))
        lpool = ctx.enter_context(tc.tile_pool(name="lpool", bufs=2))
        bpool = ctx.enter_context(tc.tile_pool(name="bpool", bufs=2))
        opool = ctx.enter_context(tc.tile_pool(name="opool", bufs=3))
        # PSUM: exactly 8 banks
        spp = ctx.enter_context(tc.tile_pool(name="spp", bufs=1, space="PSUM"))
        ctxp = ctx.enter_context(tc.tile_pool(name="ctxp", bufs=1, space="PSUM"))
        lpp = ctx.enter_context(tc.tile_pool(name="lpp", bufs=1, space="PSUM"))
        shp = ctx.enter_context(tc.tile_pool(name="shp", bufs=1, space="PSUM"))

        # ---- persistent SBUF ----
        wq_sb = singles.tile([128, 8, DPC], F16)
        wk_sb = singles.tile([128, 8, DPC], F16)
        wv_sb = singles.tile([128, 8, DPC], F16)
        wo_sb = singles.tile([128, HID], F16)
        cos_sb = singles.tile([128, S], F16)
        sin_sb = singles.tile([128, S], F16)
        qT_sb = singles.tile([128, T], F16)
        kT_sb = singles.tile([128, T], F16)
        v_all = singles.tile([128, 64, HD], F16)   # slot = h*32 + b*16 + kb
        ctx0_sb = singles.tile([128, S], F16)      # normalized ctx^T, b=0
        ctx1_sb = singles.tile([128, S], F16)
        ones_sb = singles.tile([128, 32], F16)

        nc.sync.dma_start(out=wq_sb[:].rearrange("p a b -> p (a b)"), in_=wq_d[:])
        nc.scalar.dma_start(out=wk_sb[:].rearrange("p a b -> p (a b)"), in_=wk_d[:])
        nc.gpsimd.dma_start(out=wv_sb[:].rearrange("p a b -> p (a b)"), in_=wv_d[:])
        nc.gpsimd.dma_start(out=wo_sb[:], in_=wo_d[:])
        nc.scalar.dma_start(out=cos_sb[:], in_=cos_d[:])
        nc.scalar.dma_start(out=sin_sb[:], in_=sin_d[:])
        nc.vector.memset(ones_sb[:], 1.0)
        # preload the exp table set during the lead-in
        tblw = singles.tile([1, 8], F32)
        nc.vector.memset(tblw[:], 0.0)
        nc.scalar.activation(out=tblw[:], in_=tblw[:],
                             func=mybir.ActivationFunctionType.Exp)

        # x cache: one SBUF slot per batch, [128, kc, tokens-in-batch],
        # loaded token-major in [128,512] pieces so consumers start early
        x_sb = {}

        def load_x(b, q, tcns=range(4)):
            if b not in x_sb:
                x_sb[b] = xpool.tile([128, 8, S], F16, tag="xb", name="x_sb")
            for i, tcn in enumerate(tcns):
                for kc in range(8):
                    q[(i * 8 + kc) % len(q)].dma_start(
                        out=x_sb[b][:, kc, tcn * 512:(tcn + 1) * 512],
                        in_=xT_d[kc * 128:(kc + 1) * 128,
                                 b * S + tcn * 512:b * S + (tcn + 1) * 512])

        # ---- building blocks ----
        def qk_part(kind, b, tcn, part, state):
            # half a q/k projection chunk (kc 0-3 or 4-7); state carries ps
            w_sb = wq_sb if kind == "q" else wk_sb
            t_sb = qT_sb if kind == "q" else kT_sb
            if part == 0:
                state["ps"] = shp.tile([128, 512], F32, tag="sh", name="ps")
            ps = state["ps"]
            for kc in range(part * 4, part * 4 + 4):
                nc.tensor.matmul(ps[:], w_sb[:, kc, :],
                                 x_sb[b][:, kc, tcn * 512:(tcn + 1) * 512],
                                 start=kc == 0, stop=kc == 7)
            if part == 1:
                tsl = slice(b * S + tcn * 512, b * S + (tcn + 1) * 512)
                nc.vector.tensor_copy(t_sb[:, tsl], ps[:])

        def qk_chunk(kind, b, tcn):
            st = {}
            qk_part(kind, b, tcn, 0, st)
            qk_part(kind, b, tcn, 1, st)

        def v_part(b, tcn, part, state):
            # half a v chunk (kc 0-3 or 4-7); ONE accumulation group per bank
            # (start=True clears has_written bank-wide for written partitions)
            if part == 0:
                state["ps"] = shp.tile([128, 4, 128], F32, tag="sh",
                                       name="vps")
            ps = state["ps"]
            for kc in range(part * 4, part * 4 + 4):
                for sub in range(4):
                    nc.tensor.matmul(
                        ps[:, sub, :],
                        x_sb[b][:, kc, tcn * 512 + sub * 128:
                                tcn * 512 + (sub + 1) * 128],
                        wv_sb[:, kc, :],
                        start=(kc == 0 and sub == 0),
                        stop=(kc == 7 and sub == 3),
                        skip_group_check=True)
            if part == 0:
                return
            base = b * 16 + tcn * 4
            d0 = v_all[:, base, 0:HD]
            dst = bass.AP(tensor=d0.tensor, offset=d0.offset,
                          ap=[list(d0.ap[0]), [HD, 4], [32 * HD, 2], [1, HD]])
            s0 = ps[:, 0, 0:HD]
            src = bass.AP(tensor=s0.tensor, offset=s0.offset,
                          ap=[list(s0.ap[0]), [128, 4], [HD, 2], [1, HD]])
            nc.vector.tensor_copy(dst, src)

        def v_chunk(b, tcn):
            st = {}
            v_part(b, tcn, 0, st)
            v_part(b, tcn, 1, st)

        def rope_half(t_sb, b, half, rot_eng):
            # t[:, h] = t[:, h]*cos[h] + rot(t)[:, h]*sin[h] for one
            # 1024-col half of batch b.  rot = partition swap via DMA.
            hsl = slice(half * 1024, (half + 1) * 1024)
            bsl = slice(b * S + half * 1024, b * S + (half + 1) * 1024)
            rot = rotp.tile([128, 1024], F16, tag="rot", name="rot")
            nc.sync.dma_start(out=rot[0:32, :], in_=t_sb[32:64, bsl])
            nc.gpsimd.dma_start(out=rot[32:64, :], in_=t_sb[0:32, bsl])
            nc.sync.dma_start(out=rot[64:96, :], in_=t_sb[96:128, bsl])
            nc.gpsimd.dma_start(out=rot[96:128, :], in_=t_sb[64:96, bsl])
            rot_eng.tensor_mul(rot[:], rot[:], sin_sb[:, hsl])
            nc.vector.tensor_mul(t_sb[:, bsl], t_sb[:, bsl], cos_sb[:, hsl])
            nc.vector.tensor_add(t_sb[:, bsl], t_sb[:, bsl], rot[:])

        ot_cache = {}

        def proj_unit(b, qb, oc, pool=None, tag="sh", dmaq=nc.gpsimd):
            src = ctx0_sb if b == 0 else ctx1_sb
            qsl = slice(qb * 128, (qb + 1) * 128)
            osl = slice(oc * 512, (oc + 1) * 512)
            ps = (pool or shp).tile([128, 512], F32, tag=tag, name="pps")
            nc.tensor.matmul(ps[:], src[:, qsl], wo_sb[:, osl],
                             start=True, stop=True)
            if oc == 0:
                ot_cache[(b, qb)] = opool.tile([128, HID], F16, tag="ot",
                                               name="ot")
            ot = ot_cache[(b, qb)]
            nc.vector.tensor_copy(ot[:, osl], ps[:])
            if oc == 1:
                r0 = b * S + qb * 128
                dmaq.dma_start(out=out_d[r0:r0 + 128, :], in_=ot[:])
                del ot_cache[(b, qb)]

        # ---- deadline-tagged work queue ----
        # item: (cost_ns, deadline (qi, kb), emit_fn); FIFO order preserved.
        # Each drain() call adds `budget` of PE credit; items emit when
        # enough credit accumulated OR their deadline is due.
        work = []
        credit = [0]

        def drain(now, budget):
            credit[0] = min(credit[0] + budget, 6000)
            while work:
                cost, dl, fn = work[0]
                if dl > now and credit[0] < cost:
                    break
                work.pop(0)
                fn()
                credit[0] = max(credit[0] - cost, -6000)
        END = (9, 0)

        # ---- lead-in: batch 0's qkv + ropes, DMA-pipelined per tcn ----
        with nc.named_scope("lead"):
            load_x(0, [nc.sync, nc.gpsimd, nc.scalar])
            qk_chunk("k", 0, 0)
            qk_chunk("k", 0, 1)
            qk_chunk("q", 0, 0)
            qk_chunk("q", 0, 1)
            rope_half(kT_sb, 0, 0, nc.gpsimd)
            rope_half(qT_sb, 0, 0, nc.vector)
            v_chunk(0, 0)
            v_chunk(0, 1)
            qk_chunk("k", 0, 2)
            qk_chunk("k", 0, 3)
            qk_chunk("q", 0, 2)
            qk_chunk("q", 0, 3)
            rope_half(kT_sb, 0, 1, nc.gpsimd)
            rope_half(qT_sb, 0, 1, nc.vector)
            v_chunk(0, 2)
            v_chunk(0, 3)

        # ---- queue batch 1 (x streams during Q1) + proj, with deadlines ----
        A = work.append
        CH, HF, PR, RP = 2600, 1300, 330, 1100

        def qk_items(kind, b, tcn, dl0, dl1):
            st = {}
            A((HF, dl0, lambda: qk_part(kind, b, tcn, 0, st)))
            A((HF, dl1, lambda: qk_part(kind, b, tcn, 1, st)))

        def v_items(b, tcn, dl0, dl1):
            st = {}
            A((HF, dl0, lambda: v_part(b, tcn, 0, st)))
            A((HF, dl1, lambda: v_part(b, tcn, 1, st)))

        A((200, (0, 0), lambda: load_x(1, [nc.sync, nc.gpsimd])))
        qk_items("k", 1, 0, (0, 12), (0, 14))
        qk_items("k", 1, 1, (1, 0), (1, 2))
        qk_items("k", 1, 2, (1, 4), (1, 6))
        qk_items("k", 1, 3, (1, 8), (1, 10))
        A((RP, (1, 12), lambda: rope_half(kT_sb, 1, 0, nc.gpsimd)))
        A((RP, (1, 14), lambda: rope_half(kT_sb, 1, 1, nc.gpsimd)))
        qk_items("q", 1, 0, (1, 13), (1, 15))
        qk_items("q", 1, 1, (1, 16), (1, 17))
        A((RP, (2, 0), lambda: rope_half(qT_sb, 1, 0, nc.vector)))
        v_items(1, 0, (2, 0), (2, 0))
        v_items(1, 1, (2, 2), (2, 3))
        v_items(1, 2, (2, 6), (2, 7))
        v_items(1, 3, (2, 10), (2, 11))
        qk_items("q", 1, 2, (3, 0), (3, 0))
        qk_items("q", 1, 3, (3, 0), (3, 0))
        A((RP, (3, 0), lambda: rope_half(qT_sb, 1, 1, nc.vector)))

        def queue_proj(b, qb_range):
            for qb in qb_range:
                for oc in range(2):
                    A((PR, END, lambda a=qb, o=oc, bb=b: proj_unit(bb, a, o)))

        # ---- attention quarters ----
        for qi, (b, qc) in enumerate([(0, 0), (0, 1), (1, 0), (1, 1)]):
            ctx_sb = ctx0_sb if b == 0 else ctx1_sb
            # previous quarters' proj becomes available now (Q1's deferred
            # to Q3 so Q2's slack stays for b1's q projections)
            if qi == 2:
                queue_proj(0, range(8))
                queue_proj(0, range(8, 16))
            elif qi == 3:
                queue_proj(1, range(8))
            with nc.named_scope(f"attn{qi}"):
                q0 = b * S + qc * 1024
                csl = slice(qc * 1024, (qc + 1) * 1024)
                ctx_t = ctxp.tile([128, 1024], F32, tag="ctx", name="ctx_t")
                l_t = lpp.tile([128, 512], F32, tag="l", name="l_t")
                pring = {}
                for kb in range(18):
                    # deadlines must fire BEFORE this kb's attention work is
                    # emitted (emission order defines dependency direction)
                    drain((qi, kb), 700)
                    if kb < 16:
                        k0 = b * S + kb * 128
                        ksl = slice(k0, k0 + 128)
                        sp0 = spp.tile([128, 1024], F32, tag="sp0", name="sp0")
                        sp1 = spp.tile([128, 1024], F32, tag="sp1", name="sp1")
                        for half in range(2):
                            qsl = slice(q0 + half * 512, q0 + (half + 1) * 512)
                            hsl = slice(half * 512, (half + 1) * 512)
                            nc.tensor.matmul(sp0[:, hsl], kT_sb[0:64, ksl],
                                             qT_sb[0:64, qsl],
                                             start=True, stop=True)
                            nc.tensor.matmul(sp1[:, hsl], kT_sb[64:128, ksl],
                                             qT_sb[64:128, qsl],
                                             start=True, stop=True)
                        p0 = ppool.tile([128, 1024], F16, tag="p0", name="p0")
                        p1 = ppool.tile([128, 1024], F16, tag="p1", name="p1")
                        nc.scalar.activation(
                            out=p0[:], in_=sp0[:],
                            func=mybir.ActivationFunctionType.Exp)
                        nc.scalar.activation(
                            out=p1[:], in_=sp1[:],
                            func=mybir.ActivationFunctionType.Exp)
                        pring[kb] = (p0, p1)
                        if dbg is not None and qi == 0 and kb == 0:
                            dp = opool.tile([128, 1024], F32, tag="dbgp", bufs=1)
                            nc.vector.tensor_copy(dp[:], p0[:])
                            nc.sync.dma_start(out=dbg["p0"][:], in_=dp[:])
                    if kb >= 2:
                        kv = kb - 2
                        p0, p1 = pring.pop(kv)
                        sl0 = b * 16 + kv
                        sl1 = 32 + b * 16 + kv
                        st, sp_ = kv == 0, kv == 15
                        for half in range(2):
                            hsl = slice(half * 512, (half + 1) * 512)
                            nc.tensor.matmul(ctx_t[0:64, hsl],
                                             v_all[:, sl0, :], p0[:, hsl],
                                             start=st, stop=sp_)
                            nc.tensor.matmul(ctx_t[64:128, hsl],
                                             v_all[:, sl1, :], p1[:, hsl],
                                             start=st, stop=sp_)
                        nc.tensor.matmul(l_t[0:32, :], ones_sb[:],
                                         p0[:, 0:512], start=st, stop=sp_,
                                         tile_position=(0, 0))
                        nc.tensor.matmul(l_t[32:64, :], ones_sb[:],
                                         p0[:, 512:1024], start=st, stop=sp_,
                                         tile_position=(0, 32))
                        nc.tensor.matmul(l_t[64:96, :], ones_sb[:],
                                         p1[:, 0:512], start=st, stop=sp_,
                                         tile_position=(0, 64))
                        nc.tensor.matmul(l_t[96:128, :], ones_sb[:],
                                         p1[:, 512:1024], start=st, stop=sp_,
                                         tile_position=(0, 96))

                # ---- normalize this quarter ----
                cuh = lpool.tile([128, 1024], F32, tag="cu", name="cuh")
                nc.vector.tensor_copy(cuh[:], ctx_t[:])
                if dbg is not None and qi == 0:
                    nc.sync.dma_start(out=dbg["ctxq1"][:], in_=cuh[:])
                lsb = lpool.tile([128, 512], F32, tag="ls", name="lsb")
                nc.vector.tensor_copy(lsb[:], l_t[:])
                lrec = lpool.tile([128, 512], F32, tag="lr", name="lrec")
                nc.vector.reciprocal_approx_fast(out=lrec[:], in_=lsb[:])
                if dbg is not None and qi == 0:
                    nc.sync.dma_start(out=dbg["lq"][:], in_=lrec[:])
                if dbg is not None and qi == 1:
                    nc.sync.dma_start(out=dbg["ls2"][:], in_=lsb[:])
                    nc.sync.dma_start(out=dbg["lq2"][:], in_=lrec[:])
                nq = nc.scalar if qi == 3 else nc.sync
                nq.dma_start(out=rscr_d[qi, 0:97, :], in_=lrec[0:97, :])
                bct = bpool.tile([128, 1024], F32, tag="bct", name="bct")
                for h in range(2):
                    for half in range(2):
                        rr = rscr_d[qi, (h * 2 + half) * 32, :]
                        nq.dma_start(
                            out=bct[h * 64:(h + 1) * 64,
                                    half * 512:(half + 1) * 512],
                            in_=bass.AP(tensor=rr.tensor, offset=rr.offset,
                                        ap=[[0, 64], [1, 512]]))
                if dbg is not None and qi == 0:
                    nc.sync.dma_start(out=dbg["bct1"][:], in_=bct[:])
                nc.vector.tensor_mul(ctx_sb[0:64, csl], cuh[0:64, :],
                                     bct[0:64, :])
                nc.vector.tensor_mul(ctx_sb[64:128, csl], cuh[64:128, :],
                                     bct[64:128, :])

        # ---- tail: drain leftovers, then Q4's proj over 4 psum slots ----
        with nc.named_scope("tail"):
            drain(END, 10**9)
            units = [(1, qb, oc) for qb in range(8, 16) for oc in range(2)]
            pools = [(spp, "sp0"), (spp, "sp1"), (ctxp, "ctx"), (shp, "sh")]
            for i, (b_, qb, oc) in enumerate(units):
                pl, tg = pools[i % 4]
                proj_unit(b_, qb, oc, pool=pl, tag=tg,
                          dmaq=(nc.sync, nc.gpsimd, nc.scalar)[i % 3])

        if dbg is not None:
            nc.gpsimd.dma_start(out=dbg["qT"][:], in_=qT_sb[:])
            nc.gpsimd.dma_start(out=dbg["kT"][:], in_=kT_sb[:])
            dv = opool.tile([128, 64], F32, tag="dbgv", bufs=1)
            nc.vector.tensor_copy(dv[:], v_all[:, 0, :])
            nc.sync.dma_start(out=dbg["v0"][:], in_=dv[:])
            dva = opool.tile([128, 64 * HD], F32, tag="dbgva", bufs=1)
            nc.vector.tensor_copy(dva[:], v_all[:].rearrange("p a b -> p (a b)"))
            nc.sync.dma_start(out=dbg["vall"][:], in_=dva[:])
            dc = opool.tile([128, S], F32, tag="dbgc", bufs=1)
            nc.vector.tensor_copy(dc[:], ctx0_sb[:])
            nc.sync.dma_start(out=dbg["ctx0"][:], in_=dc[:])


def _swz(w):
    # [1024, 128] -> [128, 1024]: SBUF layout [p, kc*128+d] = w[kc*128+p, d]
    return np.ascontiguousarray(
        w.reshape(8, 128, 128).transpose(1, 0, 2).reshape(128, 1024))


def _prep_inputs(x, Wq, Wk, Wv, Wo):
    x2 = np.asarray(x, dtype=np.float32).reshape(T, HID)
    xT16 = np.ascontiguousarray(x2.T).astype(np.float16)

    half = HD // 2
    inv_freq = (1.0 / (ROPE_BASE ** (np.arange(half, dtype=np.float64) * 2.0 / HD)))
    ang = np.arange(S, dtype=np.float64)[None, :] * inv_freq[:, None]  # [32, S]
    cosf = np.tile(np.cos(ang), (4, 1)).astype(np.float16)
    sgn = np.repeat([-1.0, 1.0, -1.0, 1.0], 32)[:, None]
    sins = (np.tile(np.sin(ang), (4, 1)) * sgn).astype(np.float16)

    scale = np.float32(1.0 / np.sqrt(HD))
    in_maps = []
    for c in range(NCORES):
        rows = slice(c * DPC, (c + 1) * DPC)
        in_maps.append({
            "xT16": xT16,
            "wq": _swz((Wq[rows, :] * scale).T.astype(np.float16)),
            "wk": _swz(Wk[rows, :].T.astype(np.float16)),
            "wv": _swz(Wv[rows, :].T.astype(np.float16)),
            "wo": np.ascontiguousarray(Wo[:, rows].T).astype(np.float16),
            "cosf": cosf,
            "sins": sins,
        })
    return in_maps


def _run(in_maps, trace=False):
    if "nc" not in _CACHE:
        _CACHE["nc"] = _build_program()
    nc = _CACHE["nc"]
    res = run_bass_kernel_spmd(nc, in_maps, core_ids=list(range(NCORES)),
                               trace=trace)
    acc = res.results[0]["out"].astype(np.float32).copy()
    for c in range(1, NCORES):
        acc += res.results[c]["out"].astype(np.float32)
    return acc.reshape(B, S, HID), res


def kernel(x, Wq, Wk, Wv, Wo):
    in_maps = _prep_inputs(np.asarray(x), np.asarray(Wq), np.asarray(Wk),
                           np.asarray(Wv), np.asarray(Wo))
    out, _ = _run(in_maps, trace=False)
    return out


def run_profiled(x, Wq, Wk, Wv, Wo):
    in_maps = _prep_inputs(np.asarray(x), np.asarray(Wq), np.asarray(Wk),
                           np.asarray(Wv), np.asarray(Wo))
    return _run(in_maps, trace=True)


# revision 24
# speedup vs baseline: 1.2518x; 1.0283x over previous
"""Multi-head attention with RoPE (B=2, S=2048, H=16, D=64) on 8 TRN2 cores,
tensor-parallel over heads (2 heads/core); host sums the 8 rank-128 fp16
partial outputs.

Restructured from the 252us baseline around two engine budgets:
  - ScalarE runs ONLY the 128 softmax exps ([128,1024] each, ~1.1us);
    everything else (copies, DMA issue) lives on DVE/Sync/GpSimd queues.
  - PE work cut via tile_position packing:
      * scores: h0 (rows 0-63) and h1 (rows 64-127) issue as adjacent
        K=64 matmuls on disjoint row-groups -> run concurrently.
      * PV: v has no ones column (M=64); h0 writes ctx psum partitions
        0-63, h1 partitions 64-127 (col tile (0,64)), concurrently from
        the two p streams.  start=True clears has_written for the whole
        bank but only on the written partitions, so partition-disjoint
        groups are safe (free-offset-disjoint groups are NOT - see
        v_chunk, which uses a single accumulation group per bank).
      * softmax denominators via an "l-quad": four M=1 ones-stationary
        matmuls col-tiled at psum partitions 0/32/64/96, accumulating
        sum_k exp over the kb loop.
  - Single fused timeline: lead-in computes k(b0), rope-k, q(b0) first
    half, rope-q half0, v(b0) chunk0; everything else (rest of b0,
    qkv(b1), output projection) drains from a deadline-tagged work queue
    inside the attention kb loops, paced to fill PE slack under the
    ScalarE exp cadence.
  - PSUM (8 banks): sp0 2 + sp1 2 + packed ctx 2 + l 1 + one shared slot
    (qkv-psum / v-psum / proj) 1.
  - x cached in SBUF per batch (1 HBM read), loaded in [128,512] pieces
    token-major so the first k-projection starts after ~1 piece.
  - outputs stored fp16 (host sums in fp32).
"""
import numpy as np
import ml_dtypes

import concourse.bass as bass
import concourse.mybir as mybir
import concourse.tile as tile
from concourse import bacc
from concourse.bass_utils import run_bass_kernel_spmd

F32 = mybir.dt.float32
F16 = mybir.dt.float16

B, S, HID = 2, 2048, 1024
NH, HD = 16, 64
T = B * S
NCORES = 8
HPC = NH // NCORES         # 2 heads per core
DPC = HPC * HD             # 128 context dims per core
ROPE_BASE = 10000.0

_CACHE = {}
DEBUG_TAPS = False


def _build_program():
    nc = bacc.Bacc("TRN2", target_bir_lowering=False, debug=False)

    xT_d = nc.dram_tensor("xT16", [HID, T], F16, kind="ExternalInput")
    wq_d = nc.dram_tensor("wq", [128, HID], F16, kind="ExternalInput")
    wk_d = nc.dram_tensor("wk", [128, HID], F16, kind="ExternalInput")
    wv_d = nc.dram_tensor("wv", [128, HID], F16, kind="ExternalInput")
    wo_d = nc.dram_tensor("wo", [DPC, HID], F16, kind="ExternalInput")
    cos_d = nc.dram_tensor("cosf", [128, S], F16, kind="ExternalInput")
    sin_d = nc.dram_tensor("sins", [128, S], F16, kind="ExternalInput")
    out_d = nc.dram_tensor("out", [T, HID], F16, kind="ExternalOutput")
    rscr_d = nc.dram_tensor("rscr", [4, 128, 512], F32,
                            kind="ExternalOutput" if DEBUG_TAPS else "Internal")

    dbg = None
    if DEBUG_TAPS:
        dbg = {
            "qT": nc.dram_tensor("dbg_qT", [128, T], F32, kind="ExternalOutput"),
            "kT": nc.dram_tensor("dbg_kT", [128, T], F32, kind="ExternalOutput"),
            "v0": nc.dram_tensor("dbg_v0", [128, 64], F32, kind="ExternalOutput"),
            "p0": nc.dram_tensor("dbg_p0", [128, 1024], F32, kind="ExternalOutput"),
            "lq": nc.dram_tensor("dbg_lq", [128, 512], F32, kind="ExternalOutput"),
            "ls2": nc.dram_tensor("dbg_ls2", [128, 512], F32, kind="ExternalOutput"),
            "lq2": nc.dram_tensor("dbg_lq2", [128, 512], F32, kind="ExternalOutput"),
            "ctx0": nc.dram_tensor("dbg_ctx0", [128, S], F32, kind="ExternalOutput"),
            "vall": nc.dram_tensor("dbg_vall", [128, 64 * HD], F32, kind="ExternalOutput"),
            "ctxq1": nc.dram_tensor("dbg_ctxq1", [128, 1024], F32, kind="ExternalOutput"),
            "bct1": nc.dram_tensor("dbg_bct1", [128, 1024], F32, kind="ExternalOutput"),
        }

    with tile.TileContext(nc) as tc:
        _emit(nc, tc, xT_d, wq_d, wk_d, wv_d, wo_d, cos_d, sin_d, out_d,
              rscr_d, dbg=dbg)
    nc.compile()
    return nc


def _emit(nc, tc, xT_d, wq_d, wk_d, wv_d, wo_d, cos_d, sin_d, out_d,
          rscr_d, dbg=None):
    import contextlib
    ctx = contextlib.ExitStack()
    with ctx:
        singles = ctx.enter_context(tc.tile_pool(name="singles", bufs=1))
        xpool = ctx.enter_context(tc.tile_pool(name="xpool", bufs=2))
        ppool = ctx.enter_context(tc.tile_pool(name="ppool", bufs=3))
        rotp = ctx.enter_context(tc.tile_pool(name="rotp", bufs=3))
        lpool = ctx.enter_context(tc.tile_pool(name="lpool", bufs=2))
        bpool = ctx.enter_context(tc.tile_pool(name="bpool", bufs=2))
        opool = ctx.enter_context(tc.tile_pool(name="opool", bufs=3))
        # PSUM: exactly 8 banks
        spp = ctx.enter_context(tc.tile_pool(name="spp", bufs=1, space="PSUM"))
        ctxp = ctx.enter_context(tc.tile_pool(name="ctxp", bufs=1, space="PSUM"))
        lpp = ctx.enter_context(tc.tile_pool(name="lpp", bufs=1, space="PSUM"))
        shp = ctx.enter_context(tc.tile_pool(name="shp", bufs=1, space="PSUM"))

        # ---- persistent SBUF ----
        wq_sb = singles.tile([128, 8, DPC], F16)
        wk_sb = singles.tile([128, 8, DPC], F16)
        wv_sb = singles.tile([128, 8, DPC], F16)
        wo_sb = singles.tile([128, HID], F16)
        cos_sb = singles.tile([128, S], F16)
        sin_sb = singles.tile([128, S], F16)
        qT_sb = singles.tile([128, T], F16)
        kT_sb = singles.tile([128, T], F16)
        v_all = singles.tile([128, 64, HD], F16)   # slot = h*32 + b*16 + kb
        ctx0_sb = singles.tile([128, S], F16)      # normalized ctx^T, b=0
        ctx1_sb = singles.tile([128, S], F16)
        ones_sb = singles.tile([128, 32], F16)

        nc.sync.dma_start(out=wq_sb[:].rearrange("p a b -> p (a b)"), in_=wq_d[:])
        nc.scalar.dma_start(out=wk_sb[:].rearrange("p a b -> p (a b)"), in_=wk_d[:])
        nc.gpsimd.dma_start(out=wv_sb[:].rearrange("p a b -> p (a b)"), in_=wv_d[:])
        nc.gpsimd.dma_start(out=wo_sb[:], in_=wo_d[:])
        nc.scalar.dma_start(out=cos_sb[:], in_=cos_d[:])
        nc.scalar.dma_start(out=sin_sb[:], in_=sin_d[:])
        nc.vector.memset(ones_sb[:], 1.0)
        # preload the exp table set during the lead-in
        tblw = singles.tile([1, 8], F32)
        nc.vector.memset(tblw[:], 0.0)
        nc.scalar.activation(out=tblw[:], in_=tblw[:],
                             func=mybir.ActivationFunctionType.Exp)

        # x cache: one SBUF slot per batch, [128, kc, tokens-in-batch],
        # loaded token-major in [128,512] pieces so consumers start early
        x_sb = {}

        def load_x(b, q, halves=range(2)):
            if b not in x_sb:
                x_sb[b] = xpool.tile([128, 8, S], F16, tag="xb", name="x_sb")
            for i, hf in enumerate(halves):
                for kc in range(8):
                    q[(i * 8 + kc) % len(q)].dma_start(
                        out=x_sb[b][:, kc, hf * 1024:(hf + 1) * 1024],
                        in_=xT_d[kc * 128:(kc + 1) * 128,
                                 b * S + hf * 1024:b * S + (hf + 1) * 1024])

        # ---- building blocks ----
        def qk_part(kind, b, tcn, part, state):
            # half a q/k projection chunk (kc 0-3 or 4-7); state carries ps
            w_sb = wq_sb if kind == "q" else wk_sb
            t_sb = qT_sb if kind == "q" else kT_sb
            if part == 0:
                state["ps"] = shp.tile([128, 512], F32, tag="sh", name="ps")
            ps = state["ps"]
            for kc in range(part * 4, part * 4 + 4):
                nc.tensor.matmul(ps[:], w_sb[:, kc, :],
                                 x_sb[b][:, kc, tcn * 512:(tcn + 1) * 512],
                                 start=kc == 0, stop=kc == 7)
            if part == 1:
                tsl = slice(b * S + tcn * 512, b * S + (tcn + 1) * 512)
                nc.vector.tensor_copy(t_sb[:, tsl], ps[:])

        def qk_chunk(kind, b, tcn):
            st = {}
            qk_part(kind, b, tcn, 0, st)
            qk_part(kind, b, tcn, 1, st)

        def v_part(b, tcn, part, state):
            # half a v chunk (kc 0-3 or 4-7); ONE accumulation group per bank
            # (start=True clears has_written bank-wide for written partitions)
            if part == 0:
                state["ps"] = shp.tile([128, 4, 128], F32, tag="sh",
                                       name="vps")
            ps = state["ps"]
            for kc in range(part * 4, part * 4 + 4):
                for sub in range(4):
                    nc.tensor.matmul(
                        ps[:, sub, :],
                        x_sb[b][:, kc, tcn * 512 + sub * 128:
                                tcn * 512 + (sub + 1) * 128],
                        wv_sb[:, kc, :],
                        start=(kc == 0 and sub == 0),
                        stop=(kc == 7 and sub == 3),
                        skip_group_check=True)
            if part == 0:
                return
            base = b * 16 + tcn * 4
            d0 = v_all[:, base, 0:HD]
            dst = bass.AP(tensor=d0.tensor, offset=d0.offset,
                          ap=[list(d0.ap[0]), [HD, 4], [32 * HD, 2], [1, HD]])
            s0 = ps[:, 0, 0:HD]
            src = bass.AP(tensor=s0.tensor, offset=s0.offset,
                          ap=[list(s0.ap[0]), [128, 4], [HD, 2], [1, HD]])
            nc.vector.tensor_copy(dst, src)

        def v_chunk(b, tcn):
            st = {}
            v_part(b, tcn, 0, st)
            v_part(b, tcn, 1, st)

        def rope_half(t_sb, b, half, rot_eng):
            # t[:, h] = t[:, h]*cos[h] + rot(t)[:, h]*sin[h] for one
            # 1024-col half of batch b.  rot = partition swap via DMA.
            hsl = slice(half * 1024, (half + 1) * 1024)
            bsl = slice(b * S + half * 1024, b * S + (half + 1) * 1024)
            rot = rotp.tile([128, 1024], F16, tag="rot", name="rot")
            nc.sync.dma_start(out=rot[0:32, :], in_=t_sb[32:64, bsl])
            nc.gpsimd.dma_start(out=rot[32:64, :], in_=t_sb[0:32, bsl])
            nc.sync.dma_start(out=rot[64:96, :], in_=t_sb[96:128, bsl])
            nc.gpsimd.dma_start(out=rot[96:128, :], in_=t_sb[64:96, bsl])
            rot_eng.tensor_mul(rot[:], rot[:], sin_sb[:, hsl])
            nc.vector.tensor_mul(t_sb[:, bsl], t_sb[:, bsl], cos_sb[:, hsl])
            nc.vector.tensor_add(t_sb[:, bsl], t_sb[:, bsl], rot[:])

        ot_cache = {}

        def proj_unit(b, qb, oc, pool=None, tag="sh", dmaq=nc.gpsimd):
            src = ctx0_sb if b == 0 else ctx1_sb
            qsl = slice(qb * 128, (qb + 1) * 128)
            osl = slice(oc * 512, (oc + 1) * 512)
            ps = (pool or shp).tile([128, 512], F32, tag=tag, name="pps")
            nc.tensor.matmul(ps[:], src[:, qsl], wo_sb[:, osl],
                             start=True, stop=True)
            if oc == 0:
                ot_cache[(b, qb)] = opool.tile([128, HID], F16, tag="ot",
                                               name="ot")
            ot = ot_cache[(b, qb)]
            nc.vector.tensor_copy(ot[:, osl], ps[:])
            if oc == 1:
                r0 = b * S + qb * 128
                dmaq.dma_start(out=out_d[r0:r0 + 128, :], in_=ot[:])
                del ot_cache[(b, qb)]

        # ---- deadline-tagged work queue ----
        # item: (cost_ns, deadline (qi, kb), emit_fn); FIFO order preserved.
        # Each drain() call adds `budget` of PE credit; items emit when
        # enough credit accumulated OR their deadline is due.
        work = []
        credit = [0]

        def drain(now, budget):
            credit[0] = min(credit[0] + budget, 6000)
            while work:
                cost, dl, fn = work[0]
                if dl > now and credit[0] < cost:
                    break
                work.pop(0)
                fn()
                credit[0] = max(credit[0] - cost, -6000)
        END = (9, 0)

        wgarb = singles.tile([128, 512], F16)  # PE warmup fuel
        nc.vector.memset(wgarb[:], 0.5)

        def warm(n):
            wps = spp.tile([128, 512], F32, tag="sp0", name="wps")
            for i in range(n):
                nc.tensor.matmul(wps[:], wgarb[:, 0:128], wgarb[:],
                                 start=True, stop=True, skip_group_check=True)

        # ---- lead-in: batch 0's qkv + ropes, DMA-pipelined per tcn ----
        with nc.named_scope("lead"):
            load_x(0, [nc.sync, nc.gpsimd, nc.scalar])
            warm(18)
            qk_chunk("k", 0, 0)
            qk_chunk("k", 0, 1)
            qk_chunk("q", 0, 0)
            qk_chunk("q", 0, 1)
            rope_half(kT_sb, 0, 0, nc.gpsimd)
            rope_half(qT_sb, 0, 0, nc.vector)
            v_chunk(0, 0)
            v_chunk(0, 1)
            warm(6)
            qk_chunk("k", 0, 2)
            qk_chunk("k", 0, 3)
            qk_chunk("q", 0, 2)
            qk_chunk("q", 0, 3)
            rope_half(kT_sb, 0, 1, nc.gpsimd)
            rope_half(qT_sb, 0, 1, nc.vector)
            v_chunk(0, 2)
            v_chunk(0, 3)

        # ---- queue batch 1 (x streams during Q1) + proj, with deadlines ----
        A = work.append
        CH, HF, PR, RP = 2600, 1300, 330, 1100

        def qk_items(kind, b, tcn, dl0, dl1):
            st = {}
            A((HF, dl0, lambda: qk_part(kind, b, tcn, 0, st)))
            A((HF, dl1, lambda: qk_part(kind, b, tcn, 1, st)))

        def v_items(b, tcn, dl0, dl1):
            st = {}
            A((HF, dl0, lambda: v_part(b, tcn, 0, st)))
            A((HF, dl1, lambda: v_part(b, tcn, 1, st)))

        A((200, (0, 0), lambda: load_x(1, [nc.sync, nc.gpsimd])))
        qk_items("k", 1, 0, (0, 12), (0, 14))
        qk_items("k", 1, 1, (1, 0), (1, 2))
        qk_items("k", 1, 2, (1, 4), (1, 6))
        qk_items("k", 1, 3, (1, 8), (1, 10))
        A((RP, (1, 12), lambda: rope_half(kT_sb, 1, 0, nc.gpsimd)))
        A((RP, (1, 14), lambda: rope_half(kT_sb, 1, 1, nc.gpsimd)))
        qk_items("q", 1, 0, (1, 13), (1, 15))
        qk_items("q", 1, 1, (1, 16), (1, 17))
        A((RP, (2, 0), lambda: rope_half(qT_sb, 1, 0, nc.vector)))
        v_items(1, 0, (2, 0), (2, 0))
        v_items(1, 1, (2, 2), (2, 3))
        v_items(1, 2, (2, 6), (2, 7))
        v_items(1, 3, (2, 10), (2, 11))
        qk_items("q", 1, 2, (3, 0), (3, 0))
        qk_items("q", 1, 3, (3, 0), (3, 0))
        A((RP, (3, 0), lambda: rope_half(qT_sb, 1, 1, nc.vector)))

        def queue_proj(b, qb_range, qi0, kb0, step):
            i = 0
            for qb in qb_range:
                for oc in range(2):
                    dl = (qi0, kb0 + (i * step) // 2)
                    A((PR, dl, lambda a=qb, o=oc, bb=b: proj_unit(bb, a, o)))
                    i += 1

        # ---- attention quarters ----
        for qi, (b, qc) in enumerate([(0, 0), (0, 1), (1, 0), (1, 1)]):
            ctx_sb = ctx0_sb if b == 0 else ctx1_sb
            # previous quarters' proj becomes available now (Q1's deferred
            # to Q3 so Q2's slack stays for b1's q projections)
            if qi == 2:
                queue_proj(0, range(8), 2, 1, 2)
                queue_proj(0, range(8, 16), 2, 9, 1)
            elif qi == 3:
                queue_proj(1, range(8), 3, 1, 2)
            with nc.named_scope(f"attn{qi}"):
                q0 = b * S + qc * 1024
                csl = slice(qc * 1024, (qc + 1) * 1024)
                ctx_t = ctxp.tile([128, 1024], F32, tag="ctx", name="ctx_t")
                l_t = lpp.tile([128, 512], F32, tag="l", name="l_t")
                pring = {}
                for kb in range(18):
                    # deadlines must fire BEFORE this kb's attention work is
                    # emitted (emission order defines dependency direction)
                    drain((qi, kb), 700)
                    if kb < 16:
                        k0 = b * S + kb * 128
                        ksl = slice(k0, k0 + 128)
                        sp0 = spp.tile([128, 1024], F32, tag="sp0", name="sp0")
                        sp1 = spp.tile([128, 1024], F32, tag="sp1", name="sp1")
                        for half in range(2):
                            qsl = slice(q0 + half * 512, q0 + (half + 1) * 512)
                            hsl = slice(half * 512, (half + 1) * 512)
                            nc.tensor.matmul(sp0[:, hsl], kT_sb[0:64, ksl],
                                             qT_sb[0:64, qsl],
                                             start=True, stop=True)
                            nc.tensor.matmul(sp1[:, hsl], kT_sb[64:128, ksl],
                                             qT_sb[64:128, qsl],
                                             start=True, stop=True)
                        p0 = ppool.tile([128, 1024], F16, tag="p0", name="p0")
                        p1 = ppool.tile([128, 1024], F16, tag="p1", name="p1")
                        nc.scalar.activation(
                            out=p0[:], in_=sp0[:],
                            func=mybir.ActivationFunctionType.Exp)
                        nc.scalar.activation(
                            out=p1[:], in_=sp1[:],
                            func=mybir.ActivationFunctionType.Exp)
                        pring[kb] = (p0, p1)
                        if dbg is not None and qi == 0 and kb == 0:
                            dp = opool.tile([128, 1024], F32, tag="dbgp", bufs=1)
                            nc.vector.tensor_copy(dp[:], p0[:])
                            nc.sync.dma_start(out=dbg["p0"][:], in_=dp[:])
                    if kb >= 2:
                        kv = kb - 2
                        p0, p1 = pring.pop(kv)
                        sl0 = b * 16 + kv
                        sl1 = 32 + b * 16 + kv
                        st, sp_ = kv == 0, kv == 15
                        for half in range(2):
                            hsl = slice(half * 512, (half + 1) * 512)
                            nc.tensor.matmul(ctx_t[0:64, hsl],
                                             v_all[:, sl0, :], p0[:, hsl],
                                             start=st, stop=sp_)
                            nc.tensor.matmul(ctx_t[64:128, hsl],
                                             v_all[:, sl1, :], p1[:, hsl],
                                             start=st, stop=sp_)
                        nc.tensor.matmul(l_t[0:32, :], ones_sb[:],
                                         p0[:, 0:512], start=st, stop=sp_,
                                         tile_position=(0, 0))
                        nc.tensor.matmul(l_t[32:64, :], ones_sb[:],
                                         p0[:, 512:1024], start=st, stop=sp_,
                                         tile_position=(0, 32))
                        nc.tensor.matmul(l_t[64:96, :], ones_sb[:],
                                         p1[:, 0:512], start=st, stop=sp_,
                                         tile_position=(0, 64))
                        nc.tensor.matmul(l_t[96:128, :], ones_sb[:],
                                         p1[:, 512:1024], start=st, stop=sp_,
                                         tile_position=(0, 96))

                # ---- normalize this quarter ----
                cuh = lpool.tile([128, 1024], F32, tag="cu", name="cuh")
                nc.vector.tensor_copy(cuh[:], ctx_t[:])
                if dbg is not None and qi == 0:
                    nc.sync.dma_start(out=dbg["ctxq1"][:], in_=cuh[:])
                lsb = lpool.tile([128, 512], F32, tag="ls", name="lsb")
                nc.vector.tensor_copy(lsb[:], l_t[:])
                lrec = lpool.tile([128, 512], F32, tag="lr", name="lrec")
                nc.vector.reciprocal_approx_fast(out=lrec[:], in_=lsb[:])
                if dbg is not None and qi == 0:
                    nc.sync.dma_start(out=dbg["lq"][:], in_=lrec[:])
                if dbg is not None and qi == 1:
                    nc.sync.dma_start(out=dbg["ls2"][:], in_=lsb[:])
                    nc.sync.dma_start(out=dbg["lq2"][:], in_=lrec[:])
                nq = nc.sync
                nq.dma_start(out=rscr_d[qi, 0:97, :], in_=lrec[0:97, :])
                bct = bpool.tile([128, 1024], F32, tag="bct", name="bct")
                for h in range(2):
                    for half in range(2):
                        rr = rscr_d[qi, (h * 2 + half) * 32, :]
                        nq.dma_start(
                            out=bct[h * 64:(h + 1) * 64,
                                    half * 512:(half + 1) * 512],
                            in_=bass.AP(tensor=rr.tensor, offset=rr.offset,
                                        ap=[[0, 64], [1, 512]]))
                if dbg is not None and qi == 0:
                    nc.sync.dma_start(out=dbg["bct1"][:], in_=bct[:])
                nc.vector.tensor_mul(ctx_sb[0:64, csl], cuh[0:64, :],
                                     bct[0:64, :])
                nc.vector.tensor_mul(ctx_sb[64:128, csl], cuh[64:128, :],
                                     bct[64:128, :])

        # ---- tail: drain leftovers, then Q4's proj over 4 psum slots ----
        with nc.named_scope("tail"):
            drain(END, 10**9)
            units = [(1, qb, oc) for qb in range(8, 16) for oc in range(2)]
            pools = [(spp, "sp0"), (spp, "sp1"), (ctxp, "ctx"), (shp, "sh")]
            for i, (b_, qb, oc) in enumerate(units):
                pl, tg = pools[i % 4]
                proj_unit(b_, qb, oc, pool=pl, tag=tg,
                          dmaq=(nc.gpsimd, nc.scalar)[i % 2])

        if dbg is not None:
            nc.gpsimd.dma_start(out=dbg["qT"][:], in_=qT_sb[:])
            nc.gpsimd.dma_start(out=dbg["kT"][:], in_=kT_sb[:])
            dv = opool.tile([128, 64], F32, tag="dbgv", bufs=1)
            nc.vector.tensor_copy(dv[:], v_all[:, 0, :])
            nc.sync.dma_start(out=dbg["v0"][:], in_=dv[:])
            dva = opool.tile([128, 64 * HD], F32, tag="dbgva", bufs=1)
            nc.vector.tensor_copy(dva[:], v_all[:].rearrange("p a b -> p (a b)"))
            nc.sync.dma_start(out=dbg["vall"][:], in_=dva[:])
            dc = opool.tile([128, S], F32, tag="dbgc", bufs=1)
            nc.vector.tensor_copy(dc[:], ctx0_sb[:])
            nc.sync.dma_start(out=dbg["ctx0"][:], in_=dc[:])


def _swz(w):
    # [1024, 128] -> [128, 1024]: SBUF layout [p, kc*128+d] = w[kc*128+p, d]
    return np.ascontiguousarray(
        w.reshape(8, 128, 128).transpose(1, 0, 2).reshape(128, 1024))


def _prep_inputs(x, Wq, Wk, Wv, Wo):
    x2 = np.asarray(x, dtype=np.float32).reshape(T, HID)
    xT16 = np.ascontiguousarray(x2.T).astype(np.float16)

    half = HD // 2
    inv_freq = (1.0 / (ROPE_BASE ** (np.arange(half, dtype=np.float64) * 2.0 / HD)))
    ang = np.arange(S, dtype=np.float64)[None, :] * inv_freq[:, None]  # [32, S]
    cosf = np.tile(np.cos(ang), (4, 1)).astype(np.float16)
    sgn = np.repeat([-1.0, 1.0, -1.0, 1.0], 32)[:, None]
    sins = (np.tile(np.sin(ang), (4, 1)) * sgn).astype(np.float16)

    scale = np.float32(1.0 / np.sqrt(HD))
    in_maps = []
    for c in range(NCORES):
        rows = slice(c * DPC, (c + 1) * DPC)
        in_maps.append({
            "xT16": xT16,
            "wq": _swz((Wq[rows, :] * scale).T.astype(np.float16)),
            "wk": _swz(Wk[rows, :].T.astype(np.float16)),
            "wv": _swz(Wv[rows, :].T.astype(np.float16)),
            "wo": np.ascontiguousarray(Wo[:, rows].T).astype(np.float16),
            "cosf": cosf,
            "sins": sins,
        })
    return in_maps


def _run(in_maps, trace=False):
    if "nc" not in _CACHE:
        _CACHE["nc"] = _build_program()
    nc = _CACHE["nc"]
    res = run_bass_kernel_spmd(nc, in_maps, core_ids=list(range(NCORES)),
                               trace=trace)
    acc = res.results[0]["out"].astype(np.float32).copy()
    for c in range(1, NCORES):
        acc += res.results[c]["out"].astype(np.float32)
    return acc.reshape(B, S, HID), res


def kernel(x, Wq, Wk, Wv, Wo):
    in_maps = _prep_inputs(np.asarray(x), np.asarray(Wq), np.asarray(Wk),
                           np.asarray(Wv), np.asarray(Wo))
    out, _ = _run(in_maps, trace=False)
    return out


def run_profiled(x, Wq, Wk, Wv, Wo):
    in_maps = _prep_inputs(np.asarray(x), np.asarray(Wq), np.asarray(Wk),
                           np.asarray(Wv), np.asarray(Wo))
    return _run(in_maps, trace=True)
